# revision 10
# baseline (speedup 1.0000x reference)
"""BiLSTM Trainium2 kernel.

Sharding: 8 cores = 4 batch quarters x 2 directions.
  core p: direction d = p // 4 (0=fwd, 1=bwd), batch quarter q = p % 4
  (the backward direction is the forward LSTM run on a time-reversed
  sequence; the final reduction is a max over time, which is order-invariant,
  so all 8 cores run the identical program on different data.)

Per core: 3 stacked LSTM layers over T steps, batch 32, H=256, run as a
lag-1 wavefront (layer l processes step t = tick - l), fully SBUF-resident:
  - token embeddings gathered + feature-major transposed on the host (bf16)
    and DMA'd straight into the X^T SBUF buffer (uploading the gathered
    activations [205 x 16000]x2B per core beats shipping the 40MB embedding
    table to every core and gathering on-device)
  - per tick: matmuls (weights streaming, batch-on-partition, fp32 PSUM accum)
    -> fused sigmoid/tanh on ScalarE across all active layers
    -> DVE cell-state update -> tanh(c) -> h -> PE transpose of h into
    feature-major h^T (the lhsT of the next tick's matmuls)
  - running max over t of layer-2 h^T
Final dense layers run on every core after an AllGather of the per-core maxes;
the host takes core 0's output.

Gate columns are permuted on host from TF order [i,j,f,o] to [f,i,o,j] so a
single ScalarE sigmoid covers all three sigmoid gates; when the layer-1/2
biases are all zero (the usual case) the +1.0 forget bias is applied for free
via the ScalarE activation-bias field and no per-step bias matmuls are
emitted; otherwise biases ride in an extra weight row against a ones-vector.
cap_table is folded into the layer-0 weights (one-hot @ (cap_table @ W_cap)).
"""

import sys

import numpy as np

sys.path.insert(0, "/opt/trn_rl_repo")

from contextlib import ExitStack

import concourse.bacc as bacc
import concourse.bass as bass
import concourse.mybir as mybir
import concourse.tile as tile
from concourse.bass_utils import run_bass_kernel_spmd
from concourse.masks import make_identity

FP32 = mybir.dt.float32
BF16 = mybir.dt.bfloat16

VOCAB, EMB, T_FULL, B_FULL, H, NC_OUT = 50000, 200, 500, 128, 256, 6
BQ = 32          # batch per core
G4 = 4 * H       # 1024 gate width
HALF = 512       # matmul N per PSUM bank

# gate slices after host permutation [f, i, o, j]
SL_F = slice(0, 256)
SL_I = slice(256, 512)
SL_O = slice(512, 768)
SL_J = slice(768, 1024)


def _build_program(T, with_tail=True, has_bias=True):
    """Build the single SPMD Bass program (same for every core)."""
    TOK = BQ * T                      # tokens per core

    nc = bacc.Bacc(None, target_bir_lowering=False, debug=False)

    # ---- external inputs (per-core data) ----
    xta_d = nc.dram_tensor("xta", [128, TOK], BF16, kind="ExternalInput")
    xtb_d = nc.dram_tensor("xtb", [77, TOK], BF16, kind="ExternalInput")
    w0 = nc.dram_tensor("w0", [461, G4], BF16, kind="ExternalInput")
    wrows = 513 if has_bias else 512
    w1 = nc.dram_tensor("w1", [wrows, G4], BF16, kind="ExternalInput")
    w2 = nc.dram_tensor("w2", [wrows, G4], BF16, kind="ExternalInput")
    d1w = nc.dram_tensor("d1w", [512, 64], BF16, kind="ExternalInput")
    d1b = nc.dram_tensor("d1b", [1, 64], BF16, kind="ExternalInput")
    d2w = nc.dram_tensor("d2w", [64, NC_OUT], FP32, kind="ExternalInput")
    d2b = nc.dram_tensor("d2b", [1, NC_OUT], FP32, kind="ExternalInput")
    out = nc.dram_tensor("out", [NC_OUT, B_FULL], FP32, kind="ExternalOutput")

    with tile.TileContext(nc) as tc, ExitStack() as ctx:
        const = ctx.enter_context(tc.tile_pool(name="const", bufs=1))
        wpool = ctx.enter_context(tc.tile_pool(name="wpool", bufs=1))
        xtp = ctx.enter_context(tc.tile_pool(name="xtp", bufs=1))
        state = ctx.enter_context(tc.tile_pool(name="state", bufs=1))
        gpool = ctx.enter_context(tc.tile_pool(name="gpool", bufs=3))
        zg = ctx.enter_context(tc.tile_pool(name="zg", bufs=3))
        hpool = ctx.enter_context(tc.tile_pool(name="hpool", bufs=2))
        htp = ctx.enter_context(tc.tile_pool(name="htp", bufs=2))
        dram = ctx.enter_context(tc.tile_pool(name="dram", bufs=1, space="DRAM"))

        # ---- constants ----
        id_f32 = const.tile([128, 128], FP32)
        make_identity(nc, id_f32[:])
        id_bf = const.tile([128, 128], BF16)
        nc.vector.tensor_copy(id_bf[:], id_f32[:])
        ones_bf = const.tile([1, 128], BF16)
        nc.gpsimd.memset(ones_bf[:], 1.0)
        ones_f32 = const.tile([1, 128], FP32)
        nc.gpsimd.memset(ones_f32[:], 1.0)

        # ---- load weights into SBUF ----
        def load_w(dw, rows_chunks):
            tiles = []
            r0 = 0
            for i, rs in enumerate(rows_chunks):
                t = wpool.tile([rs, G4], BF16, name=f"wt_{dw.name}_{i}")
                nc.sync.dma_start(t[:], dw[r0:r0 + rs, :])
                tiles.append(t)
                r0 += rs
            return tiles

        w0a, w0b, w0c, w0d = load_w(w0, [128, 77, 128, 128])
        if has_bias:
            w1a, w1b, w1bias, w1c, w1d = load_w(w1, [128, 128, 1, 128, 128])
            w2a, w2b, w2bias, w2c, w2d = load_w(w2, [128, 128, 1, 128, 128])
        else:
            w1a, w1b, w1c, w1d = load_w(w1, [128, 128, 128, 128])
            w2a, w2b, w2c, w2d = load_w(w2, [128, 128, 128, 128])
            w1bias = w2bias = None

        d1w_sb = []
        for c in range(4):
            t = wpool.tile([128, 64], BF16, name=f"d1w_{c}")
            nc.sync.dma_start(t[:], d1w[128 * c:128 * (c + 1), :])
            d1w_sb.append(t)
        d1b_sb = wpool.tile([1, 64], BF16)
        nc.sync.dma_start(d1b_sb[:], d1b[:, :])
        d2w_sb = wpool.tile([64, NC_OUT], FP32)
        nc.sync.dma_start(d2w_sb[:], d2w[:, :])
        d2b_sb = wpool.tile([1, NC_OUT], FP32)
        nc.sync.dma_start(d2b_sb[:], d2b[:, :])

        # ---- recurrent state ----
        c_all = state.tile([96, H], FP32)       # cell state, 3 layers x 32 batch
        nc.gpsimd.memset(c_all[:], 0.0)
        maxht = state.tile([128, 2, BQ], BF16)  # running max of layer-2 h^T
        nc.gpsimd.memset(maxht[:], -10.0)
        ht_init = state.tile([128, 2, 96], BF16)
        nc.gpsimd.memset(ht_init[:], 0.0)

        # X^T: xt_a rows = emb features 0:128
        #      xt_b rows = emb features 128:200 (72) | cap one-hot (4) | ones (1)
        # (host-gathered, host-transposed, bf16)
        xt_a = xtp.tile([128, TOK], BF16)
        xt_b = xtp.tile([77, TOK], BF16)
        nc.sync.dma_start(xt_a[:], xta_d[:, :])
        nc.sync.dma_start(xt_b[:], xtb_d[:, :])

        with tc.tile_pool(name="pz", bufs=2, space="PSUM") as pz, \
             tc.tile_pool(name="pht", bufs=2, space="PSUM") as pht:

            ht_prev = ht_init

            # per-layer lhsT chunk lists for step t of layer l
            def layer_chunks(l, t, ht):
                if l == 0:
                    return [
                        (xt_a[:, BQ * t:BQ * (t + 1)], w0a),
                        (xt_b[:, BQ * t:BQ * (t + 1)], w0b),
                        (ht[:, 0, 0:32], w0c),
                        (ht[:, 1, 0:32], w0d),
                    ]
                wa, wb, wbias, wc, wd = (
                    (w1a, w1b, w1bias, w1c, w1d) if l == 1 else
                    (w2a, w2b, w2bias, w2c, w2d))
                xs = slice(32 * (l - 1), 32 * l)
                hs = slice(32 * l, 32 * (l + 1))
                chunks = [
                    (ht[:, 0, xs], wa),
                    (ht[:, 1, xs], wb),
                    (ht[:, 0, hs], wc),
                    (ht[:, 1, hs], wd),
                ]
                if has_bias:
                    chunks.insert(2, (ones_bf[0:1, 0:32], wbias))
                return chunks

            # L0's x-part matmuls depend only on X^T; emit tick tau+1's
            # before tick tau's transposes so the in-order PE fills its
            # stall window while the ACT/DVE tail of tick tau runs
            z_tiles = {}

            def alloc_z(tau):
                zt = pz.tile([96, G4], FP32, name="z", tag="z")
                z_tiles[tau] = zt
                if tau <= T - 1:
                    for half in range(2):
                        ns = slice(HALF * half, HALF * (half + 1))
                        for k, lhsT in enumerate(
                                (xt_a[:, BQ * tau:BQ * (tau + 1)],
                                 xt_b[:, BQ * tau:BQ * (tau + 1)])):
                            rhs = (w0a, w0b)[k]
                            nc.tensor.matmul(
                                zt[0:32, ns], lhsT, rhs[:, ns],
                                start=(k == 0), stop=False,
                                skip_group_check=True)
                return zt

            alloc_z(0)

            # ---- wavefront over ticks ----
            for tau in range(T + 2):
                lo = max(0, tau - (T - 1))
                hi = min(2, tau)
                # HW: a partition range with non-zero base spans <= 32
                if lo == 0:
                    rlist = [slice(0, 32 * (hi + 1))]
                else:
                    rlist = [slice(32 * l, 32 * (l + 1))
                             for l in range(lo, hi + 1)]

                z = z_tiles.pop(tau)
                lchunks = {}
                for l in range(lo, hi + 1):
                    ch = layer_chunks(l, tau - l, ht_prev)
                    if l == 0:
                        ch = ch[2:]      # x-part chunks pre-emitted in alloc_z
                        starts = [False] * len(ch)
                    else:
                        starts = [k == 0 for k in range(len(ch))]
                    lchunks[l] = [(lhsT, rhs, st, k == len(ch) - 1)
                                  for k, ((lhsT, rhs), st) in
                                  enumerate(zip(ch, starts))]
                maxk = max(len(v) for v in lchunks.values())
                for half in range(2):
                    ns = slice(HALF * half, HALF * (half + 1))
                    # interleave layers per chunk step: consecutive matmuls
                    # target different 32-col groups -> concurrent PE tiles
                    for k in range(maxk):
                        for l in range(lo, hi + 1):
                            chunks = lchunks[l]
                            if k >= len(chunks):
                                continue
                            lhsT, rhs, st, sp = chunks[k]
                            zl = z[32 * l:32 * (l + 1), ns]
                            nc.tensor.matmul(
                                zl, lhsT, rhs[:, ns],
                                start=st, stop=sp,
                                skip_group_check=True,
                            )

                gates = zg.tile([96, G4], FP32, name="gates", tag="gates")
                t1 = zg.tile([96, H], FP32, name="t1", tag="t1")
                th = zg.tile([96, H], FP32, name="th", tag="th")
                h_all = hpool.tile([96, H], BF16, name="h_all", tag="h_all")
                for r in rlist:
                    if has_bias:
                        nc.scalar.activation(gates[r, 0:768], z[r, 0:768],
                                             mybir.ActivationFunctionType.Sigmoid)
                    else:
                        nc.scalar.activation(gates[r, SL_F], z[r, SL_F],
                                             mybir.ActivationFunctionType.Sigmoid,
                                             bias=1.0)
                        nc.scalar.activation(gates[r, 256:768], z[r, 256:768],
                                             mybir.ActivationFunctionType.Sigmoid)
                    nc.scalar.activation(gates[r, SL_J], z[r, SL_J],
                                         mybir.ActivationFunctionType.Tanh)
                    nc.vector.tensor_tensor(c_all[r], gates[r, SL_F], c_all[r],
                                            op=mybir.AluOpType.mult)
                    nc.vector.tensor_tensor(t1[r], gates[r, SL_I],
                                            gates[r, SL_J],
                                            op=mybir.AluOpType.mult)
                    nc.vector.tensor_tensor(c_all[r], c_all[r], t1[r],
                                            op=mybir.AluOpType.add)
                    nc.scalar.activation(th[r], c_all[r],
                                         mybir.ActivationFunctionType.Tanh)
                    nc.vector.tensor_tensor(h_all[r], gates[r, SL_O], th[r],
                                            op=mybir.AluOpType.mult)
                if tau < 2:
                    # zero the not-yet-active layers' rows so their h^T reads
                    # as the correct zero initial state next tick
                    for rz in range(hi + 1, 3):
                        nc.vector.memset(h_all[32 * rz:32 * (rz + 1), :], 0.0)

                if tau + 1 <= T + 1:
                    alloc_z(tau + 1)

                ht = htp.tile([128, 2, 96], BF16, name="ht", tag="ht")
                for c in range(2):
                    tp = pht.tile([128, 96], BF16, name="htpp", tag="htpp")
                    nc.tensor.transpose(tp[:], h_all[:, 128 * c:128 * (c + 1)],
                                        id_bf[0:96, 0:96])
                    nc.vector.tensor_copy(ht[:, c, :], tp[:])

                if tau >= 2:
                    nc.vector.tensor_tensor(maxht[:], maxht[:], ht[:, :, 64:96],
                                            op=mybir.AluOpType.max)
                ht_prev = ht

        if not with_tail:
            # cost-model builds stop before the collective tail; keep maxht
            # live by dumping a slice to the output tensor
            nc.gpsimd.dma_start(out[0:6, 0:32], maxht[0:6, 0, :])
        else:
            # ---- AllGather of per-core maxes; dense head on every core ----
            tc.strict_bb_all_engine_barrier()
            mh_dram = dram.tile([128, 2 * BQ], BF16)
            nc.sync.dma_start(
                mh_dram[:].rearrange("p (c rr) -> p c rr", c=2), maxht[:, :, :])
            ag = dram.tile([8 * 128, 2 * BQ], BF16)
            nc.gpsimd.collective_compute(
                "AllGather",
                mybir.AluOpType.bypass,
                replica_groups=[list(range(8))],
                ins=[mh_dram[:].opt()],
                outs=[ag[:].opt()],
            )

            # rnn^T chunk (d2, c) [128, 128]: feature f = 256*d2 + 128*c + p,
            # batch b = 32*q + rr  ->  ag[(4*d2+q)*128 + p, c*32 + rr]
            tc.strict_bb_all_engine_barrier()
            agv = ag[:].rearrange("(g p) (c rr) -> g p c rr", p=128, c=2)
            rnn_chunks = []
            for d2 in range(2):
                for c in range(2):
                    rc = gpool.tile([128, 4, 32], BF16, name=f"rnn_{d2}_{c}",
                                    tag="rnn", bufs=4)
                    nc.sync.dma_start(
                        rc[:],
                        agv[4 * d2:4 * d2 + 4, :, c, :].rearrange("g p rr -> p g rr"))
                    rnn_chunks.append(rc)

            with tc.tile_pool(name="pdense", bufs=1, space="PSUM") as pdense:
                h1t = pdense.tile([64, B_FULL], FP32)
                for k in range(4):
                    nc.tensor.matmul(
                        h1t[:], d1w_sb[k][:],
                        rnn_chunks[k][:].rearrange("p g rr -> p (g rr)"),
                        start=(k == 0), stop=False, skip_group_check=True)
                nc.tensor.matmul(h1t[:], d1b_sb[:], ones_bf[:],
                                 start=False, stop=True, skip_group_check=True)

                # elu(x) = max(x,0) + exp(min(x,0)) - 1
                m = zg.tile([64, B_FULL], FP32, name="m", tag="m")
                nc.vector.tensor_scalar_min(m[:], h1t[:], 0.0)
                e = zg.tile([64, B_FULL], FP32, name="e", tag="m")
                nc.scalar.activation(e[:], m[:], mybir.ActivationFunctionType.Exp)
                h1f = zg.tile([64, B_FULL], FP32, name="h1f", tag="m")
                nc.vector.tensor_scalar_max(h1f[:], h1t[:], 0.0)
                nc.vector.tensor_tensor(h1f[:], h1f[:], e[:], op=mybir.AluOpType.add)
                nc.vector.tensor_scalar_add(h1f[:], h1f[:], -1.0)

                o_ps = pdense.tile([NC_OUT, B_FULL], FP32)
                nc.tensor.matmul(o_ps[:], d2w_sb[:], h1f[:], start=True, stop=False,
                                 skip_group_check=True)
                nc.tensor.matmul(o_ps[:], d2b_sb[:], ones_f32[:],
                                 start=False, stop=True, skip_group_check=True)
                o_sb = zg.tile([NC_OUT, B_FULL], FP32, name="o_sb", tag="m")
                nc.scalar.activation(o_sb[:], o_ps[:],
                                     mybir.ActivationFunctionType.Sigmoid)
                nc.sync.dma_start(out[:, :], o_sb[:])

    nc.finalize()
    return nc


_NC_CACHE = {}
TRACE = False
LAST_RESULTS = None
LAST_RUN_WALL_S = None
LAST_PREP_S = None


def _get_program(T, has_bias=True):
    key = (T, has_bias)
    if key not in _NC_CACHE:
        _NC_CACHE[key] = _build_program(T, has_bias=has_bias)
    return _NC_CACHE[key]


def _gate_perm():
    # TF order [i, j, f, o] (256 each) -> [f, i, o, j]
    i = np.arange(0, 256)
    j = np.arange(256, 512)
    f = np.arange(512, 768)
    o = np.arange(768, 1024)
    return np.concatenate([f, i, o, j])


def _prep_lstm_w(W, b, cap_table, perm, layer0, has_bias):
    """Gate-permute, fold cap_table (layer 0) and forget bias, add bias row.

    When has_bias is False the +1.0 forget bias is applied on-device via the
    ScalarE activation bias, and layers 1/2 carry no bias row at all."""
    Wp = np.asarray(W, np.float32)[:, perm]
    bp = np.asarray(b, np.float32)[perm].copy()
    if has_bias:
        bp[0:256] += 1.0  # forget_bias folded into the sigmoid argument
    if layer0:
        w_emb = Wp[0:200]
        w_cap = np.asarray(cap_table, np.float32) @ Wp[200:203]  # [4, 1024]
        w_h = Wp[203:459]
        stacked = np.concatenate(
            [w_emb[0:128], w_emb[128:200], w_cap, bp[None, :], w_h], axis=0)
        assert stacked.shape[0] == 461
    elif has_bias:
        stacked = np.concatenate([Wp[0:256], bp[None, :], Wp[256:512]], axis=0)
        assert stacked.shape[0] == 513
    else:
        stacked = Wp
        assert stacked.shape[0] == 512
    return stacked


def _to_bf16(x):
    import ml_dtypes
    return np.ascontiguousarray(np.asarray(x)).astype(ml_dtypes.bfloat16)


def kernel(**inputs):
    import time as _time
    _tprep = _time.time()
    words = np.asarray(inputs["words"])
    capitals = np.asarray(inputs["capitals"])
    B, T = words.shape
    assert B == B_FULL

    perm = _gate_perm()
    cap_table = np.asarray(inputs["cap_table"], np.float32)
    # biases of layers 1/2 are usually all-zero; then the only bias is the
    # +1.0 forget bias, applied for free via the ScalarE activation bias,
    # and the per-step bias matmuls are dropped entirely
    hb = any(np.any(np.asarray(inputs[k], np.float32) != 0.0)
             for k in ("bf1", "bf2", "bb1", "bb2"))
    nc = _get_program(T, hb)

    w_by_dir = [
        [_prep_lstm_w(inputs["Wf0"], inputs["bf0"], cap_table, perm, True, hb),
         _prep_lstm_w(inputs["Wf1"], inputs["bf1"], cap_table, perm, False, hb),
         _prep_lstm_w(inputs["Wf2"], inputs["bf2"], cap_table, perm, False, hb)],
        [_prep_lstm_w(inputs["Wb0"], inputs["bb0"], cap_table, perm, True, hb),
         _prep_lstm_w(inputs["Wb1"], inputs["bb1"], cap_table, perm, False, hb),
         _prep_lstm_w(inputs["Wb2"], inputs["bb2"], cap_table, perm, False, hb)],
    ]
    w_bf = [[_to_bf16(w) for w in ws] for ws in w_by_dir]

    d1w_np = _to_bf16(inputs["d1_W"])
    d1b_np = _to_bf16(np.asarray(inputs["d1_b"])[None, :])
    d2w_np = np.ascontiguousarray(np.asarray(inputs["d2_W"], np.float32))
    d2b_np = np.ascontiguousarray(np.asarray(inputs["d2_b"], np.float32)[None, :])

    # ---- host-side embedding gather into feature-major X^T [205, T, 128] ----
    # rows: emb feats (200) | cap one-hot (4) | ones (1); col = (t, batch)
    import ml_dtypes
    emb_bf = np.asarray(inputs["embed_words"]).astype(ml_dtypes.bfloat16)
    g = emb_bf[words]                                   # [128, T, 200] bf16
    xt_full = np.empty((205, T, B), ml_dtypes.bfloat16)
    xt_full[0:200] = g.transpose(2, 1, 0)
    capT = capitals.T                                   # [T, 128]
    xt_full[200:204] = (capT[None, :, :] ==
                        np.arange(4, dtype=capitals.dtype)[:, None, None])
    xt_full[204] = np.float32(1.0)

    in_maps = []
    for p in range(8):
        d, q = p // 4, p % 4
        blk = xt_full[:, :, BQ * q:BQ * (q + 1)]        # [205, T, 32]
        if d == 1:
            blk = blk[:, ::-1, :]
        xt = np.ascontiguousarray(blk).reshape(205, -1)  # token = t*BQ + b
        in_maps.append({
            "xta": xt[0:128],
            "xtb": xt[128:205],
            "w0": w_bf[d][0],
            "w1": w_bf[d][1],
            "w2": w_bf[d][2],
            "d1w": d1w_np,
            "d1b": d1b_np,
            "d2w": d2w_np,
            "d2b": d2b_np,
        })

    global LAST_RESULTS, LAST_RUN_WALL_S, LAST_PREP_S
    LAST_PREP_S = _time.time() - _tprep
    kwargs = {}
    if TRACE:
        kwargs = dict(trace=True, trace_cores=list(range(8)))
    _t0 = _time.time()
    try:
        res = run_bass_kernel_spmd(nc, in_maps, core_ids=list(range(8)), **kwargs)
    except Exception:
        if not kwargs:
            raise
        res = run_bass_kernel_spmd(nc, in_maps, core_ids=list(range(8)))
    LAST_RUN_WALL_S = _time.time() - _t0
    LAST_RESULTS = res
    return np.ascontiguousarray(res.results[0]["out"].T.astype(np.float32))



# revision 13
# speedup vs baseline: 2.2067x; 2.2067x over previous
"""BiLSTM Trainium2 kernel.

Sharding: 8 cores = 4 batch quarters x 2 directions.
  core p: direction d = p // 4 (0=fwd, 1=bwd), batch quarter q = p % 4
  (the backward direction is the forward LSTM run on a time-reversed
  sequence; the final reduction is a max over time, which is order-invariant,
  so all 8 cores run the identical program on different data.)

Per core: 3 stacked LSTM layers over T steps, batch 32, H=256, run as a
lag-1 wavefront (layer l processes step t = tick - l), fully SBUF-resident:
  - token embeddings gathered + feature-major transposed on the host (bf16)
    and DMA'd straight into the X^T SBUF buffer (uploading the gathered
    activations [205 x 16000]x2B per core beats shipping the 40MB embedding
    table to every core and gathering on-device)
  - the steady-state ticks run in a hardware For_i loop (unroll 6) instead
    of a fully unrolled instruction stream: the program drops from ~21k to
    <1k instructions, which cuts the per-call BIR serialization + walrus
    compile from ~1.4s to ~0.2s (the wall-clock metric includes them)
  - per tick: stage the tick's X^T column block to a static buffer (DVE copy
    with a register offset; the PE's LDWEIGHTS path cannot take register
    offsets) -> matmuls (weights streaming, batch-on-partition, fp32 PSUM
    accum) -> fused sigmoid/tanh on ScalarE across all three layers
    -> DVE cell-state update -> tanh(c) -> h -> PE transpose of h into
    feature-major h^T (the lhsT of the next tick's matmuls, ping-pong pair)
  - running max over t of layer-2 h^T
Final dense layers run on every core after an AllGather of the per-core maxes;
the host takes core 0's output.

Gate columns are permuted on host from TF order [i,j,f,o] to [f,i,o,j] so a
single ScalarE sigmoid covers all three sigmoid gates; when the layer-1/2
biases are all zero (the usual case) the +1.0 forget bias is applied for free
via the ScalarE activation-bias field and no per-step bias matmuls are
emitted; otherwise biases ride in an extra weight row against a ones-vector.
cap_table is folded into the layer-0 weights (one-hot @ (cap_table @ W_cap)).
"""

import sys

import numpy as np

sys.path.insert(0, "/opt/trn_rl_repo")

from contextlib import ExitStack

import concourse.bacc as bacc
import concourse.bass as bass
import concourse.mybir as mybir
import concourse.tile as tile
from concourse.bass import ds
from concourse.bass_utils import run_bass_kernel_spmd
from concourse.masks import make_identity

FP32 = mybir.dt.float32
BF16 = mybir.dt.bfloat16

VOCAB, EMB, T_FULL, B_FULL, H, NC_OUT = 50000, 200, 500, 128, 256, 6
BQ = 32          # batch per core
G4 = 4 * H       # 1024 gate width
HALF = 512       # matmul N per PSUM bank
UNROLL = 6       # steady-state ticks per hardware-loop body

# gate slices after host permutation [f, i, o, j]
SL_F = slice(0, 256)
SL_I = slice(256, 512)
SL_O = slice(512, 768)
SL_J = slice(768, 1024)


def _build_program(T, with_tail=True, has_bias=True):
    """Build the single SPMD Bass program (same for every core)."""
    TOK = BQ * T                      # tokens per core
    assert (T - 2) % UNROLL == 0 and T % 2 == 0

    nc = bacc.Bacc(None, target_bir_lowering=False, debug=False)

    # ---- external inputs (per-core data) ----
    xta_d = nc.dram_tensor("xta", [128, TOK], BF16, kind="ExternalInput")
    xtb_d = nc.dram_tensor("xtb", [77, TOK], BF16, kind="ExternalInput")
    w0 = nc.dram_tensor("w0", [461, G4], BF16, kind="ExternalInput")
    wrows = 513 if has_bias else 512
    w1 = nc.dram_tensor("w1", [wrows, G4], BF16, kind="ExternalInput")
    w2 = nc.dram_tensor("w2", [wrows, G4], BF16, kind="ExternalInput")
    d1w = nc.dram_tensor("d1w", [512, 64], BF16, kind="ExternalInput")
    d1b = nc.dram_tensor("d1b", [1, 64], BF16, kind="ExternalInput")
    d2w = nc.dram_tensor("d2w", [64, NC_OUT], FP32, kind="ExternalInput")
    d2b = nc.dram_tensor("d2b", [1, NC_OUT], FP32, kind="ExternalInput")
    out = nc.dram_tensor("out", [NC_OUT, B_FULL], FP32, kind="ExternalOutput")

    with tile.TileContext(nc) as tc, ExitStack() as ctx:
        const = ctx.enter_context(tc.tile_pool(name="const", bufs=1))
        wpool = ctx.enter_context(tc.tile_pool(name="wpool", bufs=1))
        xtp = ctx.enter_context(tc.tile_pool(name="xtp", bufs=1))
        state = ctx.enter_context(tc.tile_pool(name="state", bufs=1))
        gpool = ctx.enter_context(tc.tile_pool(name="gpool", bufs=3))
        zg = ctx.enter_context(tc.tile_pool(name="zg", bufs=3))
        hpool = ctx.enter_context(tc.tile_pool(name="hpool", bufs=2))
        dram = ctx.enter_context(tc.tile_pool(name="dram", bufs=1, space="DRAM"))

        # ---- constants ----
        id_f32 = const.tile([128, 128], FP32)
        make_identity(nc, id_f32[:])
        id_bf = const.tile([128, 128], BF16)
        nc.vector.tensor_copy(id_bf[:], id_f32[:])
        ones_bf = const.tile([1, 128], BF16)
        nc.gpsimd.memset(ones_bf[:], 1.0)
        ones_f32 = const.tile([1, 128], FP32)
        nc.gpsimd.memset(ones_f32[:], 1.0)

        # ---- load weights into SBUF ----
        def load_w(dw, rows_chunks):
            tiles = []
            r0 = 0
            for i, rs in enumerate(rows_chunks):
                t = wpool.tile([rs, G4], BF16, name=f"wt_{dw.name}_{i}")
                nc.sync.dma_start(t[:], dw[r0:r0 + rs, :])
                tiles.append(t)
                r0 += rs
            return tiles

        w0a, w0b, w0c, w0d = load_w(w0, [128, 77, 128, 128])
        if has_bias:
            w1a, w1b, w1bias, w1c, w1d = load_w(w1, [128, 128, 1, 128, 128])
            w2a, w2b, w2bias, w2c, w2d = load_w(w2, [128, 128, 1, 128, 128])
        else:
            w1a, w1b, w1c, w1d = load_w(w1, [128, 128, 128, 128])
            w2a, w2b, w2c, w2d = load_w(w2, [128, 128, 128, 128])
            w1bias = w2bias = None

        d1w_sb = []
        for c in range(4):
            t = wpool.tile([128, 64], BF16, name=f"d1w_{c}")
            nc.sync.dma_start(t[:], d1w[128 * c:128 * (c + 1), :])
            d1w_sb.append(t)
        d1b_sb = wpool.tile([1, 64], BF16)
        nc.sync.dma_start(d1b_sb[:], d1b[:, :])
        d2w_sb = wpool.tile([64, NC_OUT], FP32)
        nc.sync.dma_start(d2w_sb[:], d2w[:, :])
        d2b_sb = wpool.tile([1, NC_OUT], FP32)
        nc.sync.dma_start(d2b_sb[:], d2b[:, :])

        # ---- recurrent state ----
        c_all = state.tile([96, H], FP32)       # cell state, 3 layers x 32 batch
        nc.gpsimd.memset(c_all[:], 0.0)
        maxht = state.tile([128, 2, BQ], BF16)  # running max of layer-2 h^T
        nc.gpsimd.memset(maxht[:], -10.0)
        # h^T ping-pong: tick tau writes ht_buf[tau % 2], reads ht_buf[1 - tau % 2]
        ht_buf = [state.tile([128, 2, 96], BF16, name=f"htbuf{p}")
                  for p in range(2)]
        nc.gpsimd.memset(ht_buf[1][:], 0.0)     # initial state read by tick 0

        # X^T: xt_a rows = emb features 0:128
        #      xt_b rows = emb features 128:200 (72) | cap one-hot (4) | ones (1)
        # (host-gathered, host-transposed, bf16)
        xt_a = xtp.tile([128, TOK], BF16)
        xt_b = xtp.tile([77, TOK], BF16)
        nc.sync.dma_start(xt_a[:], xta_d[:, :])
        nc.sync.dma_start(xt_b[:], xtb_d[:, :])

        # per-body-position staging for the current tick's X^T column block
        xstg = [(xtp.tile([128, BQ], BF16, name=f"xsa{p}"),
                 xtp.tile([77, BQ], BF16, name=f"xsb{p}"))
                for p in range(UNROLL)]

        with tc.tile_pool(name="pz", bufs=2, space="PSUM") as pz, \
             tc.tile_pool(name="pht", bufs=2, space="PSUM") as pht:

            def emit_tick(xa, xb, ht_prev, ht_cur, lo, hi, zero_tail,
                          maxupd):
                """One wavefront tick: layers lo..hi, layer l at step t=tau-l.

                xa/xb: lhsT for layer 0's x-part ([128,32] / [77,32] APs), or
                None when layer 0 is inactive.  ht_prev/ht_cur: h^T ping-pong
                buffers.  zero_tail: zero h rows above 32*(hi+1) (warmup).
                """
                lchunks = {}
                for l in range(lo, hi + 1):
                    if l == 0:
                        ch = [(xa, w0a), (xb, w0b),
                              (ht_prev[:, 0, 0:32], w0c),
                              (ht_prev[:, 1, 0:32], w0d)]
                    else:
                        wa, wb, wbias, wc, wd = (
                            (w1a, w1b, w1bias, w1c, w1d) if l == 1 else
                            (w2a, w2b, w2bias, w2c, w2d))
                        xs = slice(32 * (l - 1), 32 * l)
                        hs = slice(32 * l, 32 * (l + 1))
                        ch = [(ht_prev[:, 0, xs], wa),
                              (ht_prev[:, 1, xs], wb),
                              (ht_prev[:, 0, hs], wc),
                              (ht_prev[:, 1, hs], wd)]
                        if has_bias:
                            ch.insert(2, (ones_bf[0:1, 0:32], wbias))
                    lchunks[l] = ch

                z = pz.tile([96, G4], FP32, name="z", tag="z")
                maxk = max(len(v) for v in lchunks.values())
                for half in range(2):
                    ns = slice(HALF * half, HALF * (half + 1))
                    # interleave layers per chunk step: consecutive matmuls
                    # target different 32-col groups -> concurrent PE tiles
                    for k in range(maxk):
                        for l in range(lo, hi + 1):
                            chunks = lchunks[l]
                            if k >= len(chunks):
                                continue
                            lhsT, rhs = chunks[k]
                            zl = z[32 * l:32 * (l + 1), ns]
                            nc.tensor.matmul(
                                zl, lhsT, rhs[:, ns],
                                start=(k == 0), stop=(k == len(chunks) - 1),
                                skip_group_check=True)

                if lo == 0:
                    rlist = [slice(0, 32 * (hi + 1))]
                else:
                    rlist = [slice(32 * l, 32 * (l + 1))
                             for l in range(lo, hi + 1)]

                gates = zg.tile([96, G4], FP32, name="gates", tag="gates")
                t1 = zg.tile([96, H], FP32, name="t1", tag="t1")
                th = zg.tile([96, H], FP32, name="th", tag="th")
                h_all = hpool.tile([96, H], BF16, name="h_all", tag="h_all")
                for r in rlist:
                    if has_bias:
                        nc.scalar.activation(gates[r, 0:768], z[r, 0:768],
                                             mybir.ActivationFunctionType.Sigmoid)
                    else:
                        nc.scalar.activation(gates[r, SL_F], z[r, SL_F],
                                             mybir.ActivationFunctionType.Sigmoid,
                                             bias=1.0)
                        nc.scalar.activation(gates[r, 256:768], z[r, 256:768],
                                             mybir.ActivationFunctionType.Sigmoid)
                    nc.scalar.activation(gates[r, SL_J], z[r, SL_J],
                                         mybir.ActivationFunctionType.Tanh)
                    nc.vector.tensor_tensor(c_all[r], gates[r, SL_F], c_all[r],
                                            op=mybir.AluOpType.mult)
                    nc.vector.tensor_tensor(t1[r], gates[r, SL_I],
                                            gates[r, SL_J],
                                            op=mybir.AluOpType.mult)
                    nc.vector.tensor_tensor(c_all[r], c_all[r], t1[r],
                                            op=mybir.AluOpType.add)
                    nc.scalar.activation(th[r], c_all[r],
                                         mybir.ActivationFunctionType.Tanh)
                    nc.vector.tensor_tensor(h_all[r], gates[r, SL_O], th[r],
                                            op=mybir.AluOpType.mult)
                if zero_tail:
                    for rz in range(hi + 1, 3):
                        nc.vector.memset(h_all[32 * rz:32 * (rz + 1), :], 0.0)

                # transpose h into feature-major h^T (during warmup the zeroed
                # tail rows are transposed too so the buffer reads as the
                # correct zero initial state)
                if lo == 0 or zero_tail:
                    for c in range(2):
                        tp = pht.tile([128, 96], BF16, name="htpp", tag="htpp")
                        nc.tensor.transpose(tp[:],
                                            h_all[:, 128 * c:128 * (c + 1)],
                                            id_bf[0:96, 0:96])
                        nc.vector.tensor_copy(ht_cur[:, c, :], tp[:])
                else:
                    # epilogue: only layers lo..hi are live; 32-row groups
                    # (non-zero partition base must span <= 32)
                    for c in range(2):
                        for l in range(lo, hi + 1):
                            rs = slice(32 * l, 32 * (l + 1))
                            tp = pht.tile([128, 32], BF16, name="htpe",
                                          tag="htpp")
                            nc.tensor.transpose(tp[:],
                                                h_all[rs, 128 * c:128 * (c + 1)],
                                                id_bf[rs, rs])
                            nc.vector.tensor_copy(ht_cur[:, c, rs], tp[:])

                if maxupd:
                    nc.vector.tensor_tensor(maxht[:], maxht[:],
                                            ht_cur[:, :, 64:96],
                                            op=mybir.AluOpType.max)

            # ---- warmup ticks 0,1 (unrolled; static X^T slices) ----
            emit_tick(xt_a[:, 0:BQ], xt_b[:, 0:BQ],
                      ht_buf[1], ht_buf[0], 0, 0, True, False)
            emit_tick(xt_a[:, BQ:2 * BQ], xt_b[:, BQ:2 * BQ],
                      ht_buf[0], ht_buf[1], 0, 1, True, False)

            # ---- steady-state ticks 2..T-1 in a hardware loop ----
            # tau = 2 + i + p; i even so tau parity == p parity
            with tc.For_i(0, T - 2, UNROLL) as i:
                for p in range(UNROLL):
                    xa, xb = xstg[p]
                    nc.vector.tensor_copy(
                        xa[:], xt_a[:, ds(i * BQ + (2 + p) * BQ, BQ)])
                    nc.vector.tensor_copy(
                        xb[:], xt_b[:, ds(i * BQ + (2 + p) * BQ, BQ)])
                    emit_tick(xa[:], xb[:],
                              ht_buf[1 - p % 2], ht_buf[p % 2], 0, 2,
                              False, True)

            # ---- drain ticks T, T+1 (unrolled; layers 1..2 then 2) ----
            emit_tick(None, None, ht_buf[1], ht_buf[0], 1, 2, False, True)
            emit_tick(None, None, ht_buf[0], ht_buf[1], 2, 2, False, True)

        if not with_tail:
            # cost-model builds stop before the collective tail; keep maxht
            # live by dumping a slice to the output tensor
            nc.gpsimd.dma_start(out[0:6, 0:32], maxht[0:6, 0, :])
        else:
            # ---- AllGather of per-core maxes; dense head on every core ----
            tc.strict_bb_all_engine_barrier()
            mh_dram = dram.tile([128, 2 * BQ], BF16)
            nc.sync.dma_start(
                mh_dram[:].rearrange("p (c rr) -> p c rr", c=2), maxht[:, :, :])
            ag = dram.tile([8 * 128, 2 * BQ], BF16)
            nc.gpsimd.collective_compute(
                "AllGather",
                mybir.AluOpType.bypass,
                replica_groups=[list(range(8))],
                ins=[mh_dram[:].opt()],
                outs=[ag[:].opt()],
            )

            # rnn^T chunk (d2, c) [128, 128]: feature f = 256*d2 + 128*c + p,
            # batch b = 32*q + rr  ->  ag[(4*d2+q)*128 + p, c*32 + rr]
            tc.strict_bb_all_engine_barrier()
            agv = ag[:].rearrange("(g p) (c rr) -> g p c rr", p=128, c=2)
            rnn_chunks = []
            for d2 in range(2):
                for c in range(2):
                    rc = gpool.tile([128, 4, 32], BF16, name=f"rnn_{d2}_{c}",
                                    tag="rnn", bufs=4)
                    nc.sync.dma_start(
                        rc[:],
                        agv[4 * d2:4 * d2 + 4, :, c, :].rearrange("g p rr -> p g rr"))
                    rnn_chunks.append(rc)

            with tc.tile_pool(name="pdense", bufs=1, space="PSUM") as pdense:
                h1t = pdense.tile([64, B_FULL], FP32)
                for k in range(4):
                    nc.tensor.matmul(
                        h1t[:], d1w_sb[k][:],
                        rnn_chunks[k][:].rearrange("p g rr -> p (g rr)"),
                        start=(k == 0), stop=False, skip_group_check=True)
                nc.tensor.matmul(h1t[:], d1b_sb[:], ones_bf[:],
                                 start=False, stop=True, skip_group_check=True)

                # elu(x) = max(x,0) + exp(min(x,0)) - 1
                m = zg.tile([64, B_FULL], FP32, name="m", tag="m")
                nc.vector.tensor_scalar_min(m[:], h1t[:], 0.0)
                e = zg.tile([64, B_FULL], FP32, name="e", tag="m")
                nc.scalar.activation(e[:], m[:], mybir.ActivationFunctionType.Exp)
                h1f = zg.tile([64, B_FULL], FP32, name="h1f", tag="m")
                nc.vector.tensor_scalar_max(h1f[:], h1t[:], 0.0)
                nc.vector.tensor_tensor(h1f[:], h1f[:], e[:], op=mybir.AluOpType.add)
                nc.vector.tensor_scalar_add(h1f[:], h1f[:], -1.0)

                o_ps = pdense.tile([NC_OUT, B_FULL], FP32)
                nc.tensor.matmul(o_ps[:], d2w_sb[:], h1f[:], start=True, stop=False,
                                 skip_group_check=True)
                nc.tensor.matmul(o_ps[:], d2b_sb[:], ones_f32[:],
                                 start=False, stop=True, skip_group_check=True)
                o_sb = zg.tile([NC_OUT, B_FULL], FP32, name="o_sb", tag="m")
                nc.scalar.activation(o_sb[:], o_ps[:],
                                     mybir.ActivationFunctionType.Sigmoid)
                nc.sync.dma_start(out[:, :], o_sb[:])

    nc.finalize()
    return nc


_NC_CACHE = {}
TRACE = False
LAST_RESULTS = None
LAST_RUN_WALL_S = None
LAST_PREP_S = None


def _get_program(T, has_bias=True):
    key = (T, has_bias)
    if key not in _NC_CACHE:
        _NC_CACHE[key] = _build_program(T, has_bias=has_bias)
    return _NC_CACHE[key]


def _gate_perm():
    # TF order [i, j, f, o] (256 each) -> [f, i, o, j]
    i = np.arange(0, 256)
    j = np.arange(256, 512)
    f = np.arange(512, 768)
    o = np.arange(768, 1024)
    return np.concatenate([f, i, o, j])


def _prep_lstm_w(W, b, cap_table, perm, layer0, has_bias):
    """Gate-permute, fold cap_table (layer 0) and forget bias, add bias row.

    When has_bias is False the +1.0 forget bias is applied on-device via the
    ScalarE activation bias, and layers 1/2 carry no bias row at all."""
    Wp = np.asarray(W, np.float32)[:, perm]
    bp = np.asarray(b, np.float32)[perm].copy()
    if has_bias:
        bp[0:256] += 1.0  # forget_bias folded into the sigmoid argument
    if layer0:
        w_emb = Wp[0:200]
        w_cap = np.asarray(cap_table, np.float32) @ Wp[200:203]  # [4, 1024]
        w_h = Wp[203:459]
        stacked = np.concatenate(
            [w_emb[0:128], w_emb[128:200], w_cap, bp[None, :], w_h], axis=0)
        assert stacked.shape[0] == 461
    elif has_bias:
        stacked = np.concatenate([Wp[0:256], bp[None, :], Wp[256:512]], axis=0)
        assert stacked.shape[0] == 513
    else:
        stacked = Wp
        assert stacked.shape[0] == 512
    return stacked


def _to_bf16(x):
    import ml_dtypes
    return np.ascontiguousarray(np.asarray(x)).astype(ml_dtypes.bfloat16)


def kernel(**inputs):
    import time as _time
    _tprep = _time.time()
    words = np.asarray(inputs["words"])
    capitals = np.asarray(inputs["capitals"])
    B, T = words.shape
    assert B == B_FULL

    perm = _gate_perm()
    cap_table = np.asarray(inputs["cap_table"], np.float32)
    # biases of layers 1/2 are usually all-zero; then the only bias is the
    # +1.0 forget bias, applied for free via the ScalarE activation bias,
    # and the per-step bias matmuls are dropped entirely
    hb = any(np.any(np.asarray(inputs[k], np.float32) != 0.0)
             for k in ("bf1", "bf2", "bb1", "bb2"))
    nc = _get_program(T, hb)

    w_by_dir = [
        [_prep_lstm_w(inputs["Wf0"], inputs["bf0"], cap_table, perm, True, hb),
         _prep_lstm_w(inputs["Wf1"], inputs["bf1"], cap_table, perm, False, hb),
         _prep_lstm_w(inputs["Wf2"], inputs["bf2"], cap_table, perm, False, hb)],
        [_prep_lstm_w(inputs["Wb0"], inputs["bb0"], cap_table, perm, True, hb),
         _prep_lstm_w(inputs["Wb1"], inputs["bb1"], cap_table, perm, False, hb),
         _prep_lstm_w(inputs["Wb2"], inputs["bb2"], cap_table, perm, False, hb)],
    ]
    w_bf = [[_to_bf16(w) for w in ws] for ws in w_by_dir]

    d1w_np = _to_bf16(inputs["d1_W"])
    d1b_np = _to_bf16(np.asarray(inputs["d1_b"])[None, :])
    d2w_np = np.ascontiguousarray(np.asarray(inputs["d2_W"], np.float32))
    d2b_np = np.ascontiguousarray(np.asarray(inputs["d2_b"], np.float32)[None, :])

    # ---- host-side embedding gather into feature-major X^T [205, T, 128] ----
    # rows: emb feats (200) | cap one-hot (4) | ones (1); col = (t, batch)
    import ml_dtypes
    emb_bf = np.asarray(inputs["embed_words"]).astype(ml_dtypes.bfloat16)
    g = emb_bf[words]                                   # [128, T, 200] bf16
    xt_full = np.empty((205, T, B), ml_dtypes.bfloat16)
    xt_full[0:200] = g.transpose(2, 1, 0)
    capT = capitals.T                                   # [T, 128]
    xt_full[200:204] = (capT[None, :, :] ==
                        np.arange(4, dtype=capitals.dtype)[:, None, None])
    xt_full[204] = np.float32(1.0)

    in_maps = []
    for p in range(8):
        d, q = p // 4, p % 4
        blk = xt_full[:, :, BQ * q:BQ * (q + 1)]        # [205, T, 32]
        if d == 1:
            blk = blk[:, ::-1, :]
        xt = np.ascontiguousarray(blk).reshape(205, -1)  # token = t*BQ + b
        in_maps.append({
            "xta": xt[0:128],
            "xtb": xt[128:205],
            "w0": w_bf[d][0],
            "w1": w_bf[d][1],
            "w2": w_bf[d][2],
            "d1w": d1w_np,
            "d1b": d1b_np,
            "d2w": d2w_np,
            "d2b": d2b_np,
        })

    global LAST_RESULTS, LAST_RUN_WALL_S, LAST_PREP_S
    LAST_PREP_S = _time.time() - _tprep
    kwargs = {}
    if TRACE:
        kwargs = dict(trace=True, trace_cores=list(range(8)))
    _t0 = _time.time()
    try:
        res = run_bass_kernel_spmd(nc, in_maps, core_ids=list(range(8)), **kwargs)
    except Exception:
        if not kwargs:
            raise
        res = run_bass_kernel_spmd(nc, in_maps, core_ids=list(range(8)))
    LAST_RUN_WALL_S = _time.time() - _t0
    LAST_RESULTS = res
    return np.ascontiguousarray(res.results[0]["out"].T.astype(np.float32))


# revision 19
# speedup vs baseline: 3.2057x; 1.4527x over previous
"""BiLSTM Trainium2 kernel.

Sharding: 8 cores = 4 batch quarters x 2 directions.
  core p: direction d = p // 4 (0=fwd, 1=bwd), batch quarter q = p % 4
  (the backward direction is the forward LSTM run on a time-reversed
  sequence; the final reduction is a max over time, which is order-invariant,
  so all 8 cores run the identical program on different data.)

Per core: 3 stacked LSTM layers over T steps, batch 32, H=256, run as a
lag-1 wavefront (layer l processes step t = tick - l), fully SBUF-resident:
  - token embeddings gathered + feature-major transposed on the host (bf16)
    and DMA'd straight into the X^T SBUF buffer (uploading the gathered
    activations [205 x 16000]x2B per core beats shipping the 40MB embedding
    table to every core and gathering on-device)
  - the steady-state ticks run in a hardware For_i loop (unroll 6) instead
    of a fully unrolled instruction stream: the program drops from ~21k to
    <1k instructions, which cuts the per-call BIR serialization + walrus
    compile from ~1.4s to ~0.2s (the wall-clock metric includes them)
  - per tick: stage the tick's X^T column block to a static buffer (DVE copy
    with a register offset; the PE's LDWEIGHTS path cannot take register
    offsets) -> matmuls (weights streaming, batch-on-partition, fp32 PSUM
    accum) -> fused sigmoid/tanh on ScalarE across all three layers
    -> DVE cell-state update -> tanh(c) -> h -> PE transpose of h into
    feature-major h^T (the lhsT of the next tick's matmuls, ping-pong pair)
  - running max over t of layer-2 h^T
Final dense layers run on every core after an AllGather of the per-core maxes;
the host takes core 0's output.

Gate columns are permuted on host from TF order [i,j,f,o] to [f,i,o,j] so a
single ScalarE sigmoid covers all three sigmoid gates; when the layer-1/2
biases are all zero (the usual case) the +1.0 forget bias is applied for free
via the ScalarE activation-bias field and no per-step bias matmuls are
emitted; otherwise biases ride in an extra weight row against a ones-vector.
cap_table is folded into the layer-0 weights (one-hot @ (cap_table @ W_cap)).
"""

import sys

import numpy as np

sys.path.insert(0, "/opt/trn_rl_repo")

from contextlib import ExitStack

import concourse.bacc as bacc
import concourse.bass as bass
import concourse.mybir as mybir
import concourse.tile as tile
from concourse.bass import ds, IndirectOffsetOnAxis
from concourse.bass_utils import run_bass_kernel_spmd
from concourse.masks import make_identity

FP32 = mybir.dt.float32
BF16 = mybir.dt.bfloat16
INT32 = mybir.dt.int32

VOCAB, EMB, T_FULL, B_FULL, H, NC_OUT = 50000, 200, 500, 128, 256, 6
BQ = 32          # batch per core
G4 = 4 * H       # 1024 gate width
HALF = 512       # matmul N per PSUM bank
UNROLL = 6       # steady-state ticks per hardware-loop body

# gate slices after host permutation [f, i, o, j]
SL_F = slice(0, 256)
SL_I = slice(256, 512)
SL_O = slice(512, 768)
SL_J = slice(768, 1024)


def _build_program(T, with_tail=True, has_bias=True):
    """Build the single SPMD Bass program (same for every core)."""
    TOK = BQ * T                      # tokens per core
    assert (T - 2) % UNROLL == 0 and T % 2 == 0

    nc = bacc.Bacc(None, target_bir_lowering=False, debug=False)

    # ---- external inputs (per-core data) ----
    NTILE = TOK // 128                # 128-token gather tiles
    assert TOK % 128 == 0 and VOCAB % 8 == 0
    VSH = VOCAB // 8                  # vocab shard per core
    embsh = nc.dram_tensor("embsh", [VSH, EMB], BF16, kind="ExternalInput")
    widx = nc.dram_tensor("widx", [128, NTILE], INT32, kind="ExternalInput")
    caph = nc.dram_tensor("caph", [5, TOK], BF16, kind="ExternalInput")
    w0 = nc.dram_tensor("w0", [461, G4], BF16, kind="ExternalInput")
    wrows = 513 if has_bias else 512
    w1 = nc.dram_tensor("w1", [wrows, G4], BF16, kind="ExternalInput")
    w2 = nc.dram_tensor("w2", [wrows, G4], BF16, kind="ExternalInput")
    d1w = nc.dram_tensor("d1w", [512, 64], BF16, kind="ExternalInput")
    d1b = nc.dram_tensor("d1b", [1, 64], BF16, kind="ExternalInput")
    d2w = nc.dram_tensor("d2w", [64, NC_OUT], FP32, kind="ExternalInput")
    d2b = nc.dram_tensor("d2b", [1, NC_OUT], FP32, kind="ExternalInput")
    out = nc.dram_tensor("out", [NC_OUT, B_FULL], FP32, kind="ExternalOutput")

    with tile.TileContext(nc) as tc, ExitStack() as ctx:
        const = ctx.enter_context(tc.tile_pool(name="const", bufs=1))
        wpool = ctx.enter_context(tc.tile_pool(name="wpool", bufs=1))
        xtp = ctx.enter_context(tc.tile_pool(name="xtp", bufs=1))
        state = ctx.enter_context(tc.tile_pool(name="state", bufs=1))
        gpool = ctx.enter_context(tc.tile_pool(name="gpool", bufs=3))
        zg = ctx.enter_context(tc.tile_pool(name="zg", bufs=3))
        hpool = ctx.enter_context(tc.tile_pool(name="hpool", bufs=2))
        dram = ctx.enter_context(tc.tile_pool(name="dram", bufs=1, space="DRAM"))

        # ---- constants ----
        id_f32 = const.tile([128, 128], FP32)
        make_identity(nc, id_f32[:])
        id_bf = const.tile([128, 128], BF16)
        nc.vector.tensor_copy(id_bf[:], id_f32[:])
        ones_bf = const.tile([1, 128], BF16)
        nc.gpsimd.memset(ones_bf[:], 1.0)
        ones_f32 = const.tile([1, 128], FP32)
        nc.gpsimd.memset(ones_f32[:], 1.0)

        # ---- load weights into SBUF ----
        def load_w(dw, rows_chunks):
            tiles = []
            r0 = 0
            for i, rs in enumerate(rows_chunks):
                t = wpool.tile([rs, G4], BF16, name=f"wt_{dw.name}_{i}")
                nc.sync.dma_start(t[:], dw[r0:r0 + rs, :])
                tiles.append(t)
                r0 += rs
            return tiles

        w0a, w0b, w0c, w0d = load_w(w0, [128, 77, 128, 128])
        if has_bias:
            w1a, w1b, w1bias, w1c, w1d = load_w(w1, [128, 128, 1, 128, 128])
            w2a, w2b, w2bias, w2c, w2d = load_w(w2, [128, 128, 1, 128, 128])
        else:
            w1a, w1b, w1c, w1d = load_w(w1, [128, 128, 128, 128])
            w2a, w2b, w2c, w2d = load_w(w2, [128, 128, 128, 128])
            w1bias = w2bias = None

        d1w_sb = []
        for c in range(4):
            t = wpool.tile([128, 64], BF16, name=f"d1w_{c}")
            nc.sync.dma_start(t[:], d1w[128 * c:128 * (c + 1), :])
            d1w_sb.append(t)
        d1b_sb = wpool.tile([1, 64], BF16)
        nc.sync.dma_start(d1b_sb[:], d1b[:, :])
        d2w_sb = wpool.tile([64, NC_OUT], FP32)
        nc.sync.dma_start(d2w_sb[:], d2w[:, :])
        d2b_sb = wpool.tile([1, NC_OUT], FP32)
        nc.sync.dma_start(d2b_sb[:], d2b[:, :])

        # ---- recurrent state ----
        c_all = state.tile([96, H], FP32)       # cell state, 3 layers x 32 batch
        nc.gpsimd.memset(c_all[:], 0.0)
        maxht = state.tile([128, 2, BQ], BF16)  # running max of layer-2 h^T
        nc.gpsimd.memset(maxht[:], -10.0)
        # h^T ping-pong: tick tau writes ht_buf[tau % 2], reads ht_buf[1 - tau % 2]
        ht_buf = [state.tile([128, 2, 96], BF16, name=f"htbuf{p}")
                  for p in range(2)]
        nc.gpsimd.memset(ht_buf[1][:], 0.0)     # initial state read by tick 0

        # X^T: xt_a rows = emb features 0:128
        #      xt_b rows = emb features 128:200 (72) | cap one-hot (4) | ones (1)
        # Each core uploads a 1/8 vocab shard (2.5MB instead of the 6.5MB
        # pre-gathered activations); an on-device AllGather reassembles the
        # full bf16 table in DRAM and a hardware-looped indirect-DMA gather
        # + PE transpose builds X^T on-core. Upload is the wall-clock
        # bottleneck, on-device gather is ~free.
        xt_a = xtp.tile([128, TOK], BF16)
        xt_b = xtp.tile([77, TOK], BF16)
        nc.sync.dma_start(xt_b[72:77, :], caph[:, :])

        widx_sb = const.tile([128, NTILE], INT32)
        nc.sync.dma_start(widx_sb[:], widx[:, :])

        embsh_int = dram.tile([VSH, EMB], BF16)
        nc.sync.dma_start(embsh_int[:], embsh[:, :])
        emb_full = dram.tile([VOCAB, EMB], BF16, addr_space="Shared")
        nc.gpsimd.collective_compute(
            "AllGather",
            mybir.AluOpType.bypass,
            replica_groups=[list(range(8))],
            ins=[embsh_int[:].opt()],
            outs=[emb_full[:].opt()],
        )

        GU = 5                         # gather-loop unroll
        assert NTILE % GU == 0
        with tc.tile_pool(name="pprep", bufs=2, space="PSUM") as pprep:
            widx_stg = [const.tile([128, 1], INT32, name=f"wix{p}")
                        for p in range(GU)]
            with tc.For_i(0, NTILE, GU) as j:
                for p in range(GU):
                    nc.vector.tensor_copy(widx_stg[p][:],
                                          widx_sb[:, ds(j + p, 1)])
                    g = gpool.tile([128, EMB], BF16, name="gemb", tag="gemb")
                    nc.gpsimd.indirect_dma_start(
                        out=g[:],
                        out_offset=None,
                        in_=emb_full[:],
                        in_offset=IndirectOffsetOnAxis(ap=widx_stg[p][:],
                                                       axis=0),
                    )
                    tp1 = pprep.tile([128, 128], BF16, name="tp1", tag="tp")
                    nc.tensor.transpose(tp1[:], g[:, 0:128], id_bf[:])
                    nc.vector.tensor_copy(
                        xt_a[:, ds(j * 128 + p * 128, 128)], tp1[:])
                    tp2 = pprep.tile([72, 128], BF16, name="tp2", tag="tp")
                    nc.tensor.transpose(tp2[:], g[:, 128:200], id_bf[:])
                    nc.vector.tensor_copy(
                        xt_b[0:72, ds(j * 128 + p * 128, 128)], tp2[:])

        # per-body-position staging for the current tick's X^T column block
        xstg = [(xtp.tile([128, BQ], BF16, name=f"xsa{p}"),
                 xtp.tile([77, BQ], BF16, name=f"xsb{p}"))
                for p in range(UNROLL)]

        with tc.tile_pool(name="pz", bufs=2, space="PSUM") as pz, \
             tc.tile_pool(name="pht", bufs=2, space="PSUM") as pht:

            def emit_tick(xa, xb, ht_prev, ht_cur, lo, hi, zero_tail,
                          maxupd):
                """One wavefront tick: layers lo..hi, layer l at step t=tau-l.

                xa/xb: lhsT for layer 0's x-part ([128,32] / [77,32] APs), or
                None when layer 0 is inactive.  ht_prev/ht_cur: h^T ping-pong
                buffers.  zero_tail: zero h rows above 32*(hi+1) (warmup).
                """
                lchunks = {}
                for l in range(lo, hi + 1):
                    if l == 0:
                        ch = [(xa, w0a), (xb, w0b),
                              (ht_prev[:, 0, 0:32], w0c),
                              (ht_prev[:, 1, 0:32], w0d)]
                    else:
                        wa, wb, wbias, wc, wd = (
                            (w1a, w1b, w1bias, w1c, w1d) if l == 1 else
                            (w2a, w2b, w2bias, w2c, w2d))
                        xs = slice(32 * (l - 1), 32 * l)
                        hs = slice(32 * l, 32 * (l + 1))
                        ch = [(ht_prev[:, 0, xs], wa),
                              (ht_prev[:, 1, xs], wb),
                              (ht_prev[:, 0, hs], wc),
                              (ht_prev[:, 1, hs], wd)]
                        if has_bias:
                            ch.insert(2, (ones_bf[0:1, 0:32], wbias))
                    lchunks[l] = ch

                z = pz.tile([96, G4], FP32, name="z", tag="z")
                maxk = max(len(v) for v in lchunks.values())
                for half in range(2):
                    ns = slice(HALF * half, HALF * (half + 1))
                    # interleave layers per chunk step: consecutive matmuls
                    # target different 32-col groups -> concurrent PE tiles
                    for k in range(maxk):
                        for l in range(lo, hi + 1):
                            chunks = lchunks[l]
                            if k >= len(chunks):
                                continue
                            lhsT, rhs = chunks[k]
                            zl = z[32 * l:32 * (l + 1), ns]
                            nc.tensor.matmul(
                                zl, lhsT, rhs[:, ns],
                                start=(k == 0), stop=(k == len(chunks) - 1),
                                skip_group_check=True)

                if lo == 0:
                    rlist = [slice(0, 32 * (hi + 1))]
                else:
                    rlist = [slice(32 * l, 32 * (l + 1))
                             for l in range(lo, hi + 1)]

                gates = zg.tile([96, G4], FP32, name="gates", tag="gates")
                t1 = zg.tile([96, H], FP32, name="t1", tag="t1")
                th = zg.tile([96, H], FP32, name="th", tag="th")
                h_all = hpool.tile([96, H], BF16, name="h_all", tag="h_all")
                for r in rlist:
                    if has_bias:
                        nc.scalar.activation(gates[r, 0:768], z[r, 0:768],
                                             mybir.ActivationFunctionType.Sigmoid)
                    else:
                        nc.scalar.activation(gates[r, SL_F], z[r, SL_F],
                                             mybir.ActivationFunctionType.Sigmoid,
                                             bias=1.0)
                        nc.scalar.activation(gates[r, 256:768], z[r, 256:768],
                                             mybir.ActivationFunctionType.Sigmoid)
                    nc.scalar.activation(gates[r, SL_J], z[r, SL_J],
                                         mybir.ActivationFunctionType.Tanh)
                    nc.vector.tensor_tensor(c_all[r], gates[r, SL_F], c_all[r],
                                            op=mybir.AluOpType.mult)
                    nc.vector.tensor_tensor(t1[r], gates[r, SL_I],
                                            gates[r, SL_J],
                                            op=mybir.AluOpType.mult)
                    nc.vector.tensor_tensor(c_all[r], c_all[r], t1[r],
                                            op=mybir.AluOpType.add)
                    nc.scalar.activation(th[r], c_all[r],
                                         mybir.ActivationFunctionType.Tanh)
                    nc.vector.tensor_tensor(h_all[r], gates[r, SL_O], th[r],
                                            op=mybir.AluOpType.mult)
                if zero_tail:
                    for rz in range(hi + 1, 3):
                        nc.vector.memset(h_all[32 * rz:32 * (rz + 1), :], 0.0)

                # transpose h into feature-major h^T (during warmup the zeroed
                # tail rows are transposed too so the buffer reads as the
                # correct zero initial state)
                if lo == 0 or zero_tail:
                    for c in range(2):
                        tp = pht.tile([128, 96], BF16, name="htpp", tag="htpp")
                        nc.tensor.transpose(tp[:],
                                            h_all[:, 128 * c:128 * (c + 1)],
                                            id_bf[0:96, 0:96])
                        nc.vector.tensor_copy(ht_cur[:, c, :], tp[:])
                else:
                    # epilogue: only layers lo..hi are live; 32-row groups
                    # (non-zero partition base must span <= 32)
                    for c in range(2):
                        for l in range(lo, hi + 1):
                            rs = slice(32 * l, 32 * (l + 1))
                            tp = pht.tile([128, 32], BF16, name="htpe",
                                          tag="htpp")
                            nc.tensor.transpose(tp[:],
                                                h_all[rs, 128 * c:128 * (c + 1)],
                                                id_bf[rs, rs])
                            nc.vector.tensor_copy(ht_cur[:, c, rs], tp[:])

                if maxupd:
                    nc.vector.tensor_tensor(maxht[:], maxht[:],
                                            ht_cur[:, :, 64:96],
                                            op=mybir.AluOpType.max)

            # ---- warmup ticks 0,1 (unrolled; static X^T slices) ----
            emit_tick(xt_a[:, 0:BQ], xt_b[:, 0:BQ],
                      ht_buf[1], ht_buf[0], 0, 0, True, False)
            emit_tick(xt_a[:, BQ:2 * BQ], xt_b[:, BQ:2 * BQ],
                      ht_buf[0], ht_buf[1], 0, 1, True, False)

            # ---- steady-state ticks 2..T-1 in a hardware loop ----
            # tau = 2 + i + p; i even so tau parity == p parity
            with tc.For_i(0, T - 2, UNROLL) as i:
                for p in range(UNROLL):
                    xa, xb = xstg[p]
                    nc.vector.tensor_copy(
                        xa[:], xt_a[:, ds(i * BQ + (2 + p) * BQ, BQ)])
                    nc.vector.tensor_copy(
                        xb[:], xt_b[:, ds(i * BQ + (2 + p) * BQ, BQ)])
                    emit_tick(xa[:], xb[:],
                              ht_buf[1 - p % 2], ht_buf[p % 2], 0, 2,
                              False, True)

            # ---- drain ticks T, T+1 (unrolled; layers 1..2 then 2) ----
            emit_tick(None, None, ht_buf[1], ht_buf[0], 1, 2, False, True)
            emit_tick(None, None, ht_buf[0], ht_buf[1], 2, 2, False, True)

        if not with_tail:
            # cost-model builds stop before the collective tail; keep maxht
            # live by dumping a slice to the output tensor
            nc.gpsimd.dma_start(out[0:6, 0:32], maxht[0:6, 0, :])
        else:
            # ---- AllGather of per-core maxes; dense head on every core ----
            tc.strict_bb_all_engine_barrier()
            mh_dram = dram.tile([128, 2 * BQ], BF16)
            nc.sync.dma_start(
                mh_dram[:].rearrange("p (c rr) -> p c rr", c=2), maxht[:, :, :])
            ag = dram.tile([8 * 128, 2 * BQ], BF16)
            nc.gpsimd.collective_compute(
                "AllGather",
                mybir.AluOpType.bypass,
                replica_groups=[list(range(8))],
                ins=[mh_dram[:].opt()],
                outs=[ag[:].opt()],
            )

            # rnn^T chunk (d2, c) [128, 128]: feature f = 256*d2 + 128*c + p,
            # batch b = 32*q + rr  ->  ag[(4*d2+q)*128 + p, c*32 + rr]
            tc.strict_bb_all_engine_barrier()
            agv = ag[:].rearrange("(g p) (c rr) -> g p c rr", p=128, c=2)
            rnn_chunks = []
            for d2 in range(2):
                for c in range(2):
                    rc = gpool.tile([128, 4, 32], BF16, name=f"rnn_{d2}_{c}",
                                    tag="rnn", bufs=4)
                    nc.sync.dma_start(
                        rc[:],
                        agv[4 * d2:4 * d2 + 4, :, c, :].rearrange("g p rr -> p g rr"))
                    rnn_chunks.append(rc)

            with tc.tile_pool(name="pdense", bufs=1, space="PSUM") as pdense:
                h1t = pdense.tile([64, B_FULL], FP32)
                for k in range(4):
                    nc.tensor.matmul(
                        h1t[:], d1w_sb[k][:],
                        rnn_chunks[k][:].rearrange("p g rr -> p (g rr)"),
                        start=(k == 0), stop=False, skip_group_check=True)
                nc.tensor.matmul(h1t[:], d1b_sb[:], ones_bf[:],
                                 start=False, stop=True, skip_group_check=True)

                # elu(x) = max(x,0) + exp(min(x,0)) - 1
                m = zg.tile([64, B_FULL], FP32, name="m", tag="m")
                nc.vector.tensor_scalar_min(m[:], h1t[:], 0.0)
                e = zg.tile([64, B_FULL], FP32, name="e", tag="m")
                nc.scalar.activation(e[:], m[:], mybir.ActivationFunctionType.Exp)
                h1f = zg.tile([64, B_FULL], FP32, name="h1f", tag="m")
                nc.vector.tensor_scalar_max(h1f[:], h1t[:], 0.0)
                nc.vector.tensor_tensor(h1f[:], h1f[:], e[:], op=mybir.AluOpType.add)
                nc.vector.tensor_scalar_add(h1f[:], h1f[:], -1.0)

                o_ps = pdense.tile([NC_OUT, B_FULL], FP32)
                nc.tensor.matmul(o_ps[:], d2w_sb[:], h1f[:], start=True, stop=False,
                                 skip_group_check=True)
                nc.tensor.matmul(o_ps[:], d2b_sb[:], ones_f32[:],
                                 start=False, stop=True, skip_group_check=True)
                o_sb = zg.tile([NC_OUT, B_FULL], FP32, name="o_sb", tag="m")
                nc.scalar.activation(o_sb[:], o_ps[:],
                                     mybir.ActivationFunctionType.Sigmoid)
                nc.sync.dma_start(out[:, :], o_sb[:])

    nc.finalize()
    return nc


_NC_CACHE = {}
TRACE = False
LAST_RESULTS = None
LAST_RUN_WALL_S = None
LAST_PREP_S = None


def _get_program(T, has_bias=True):
    key = (T, has_bias)
    if key not in _NC_CACHE:
        _NC_CACHE[key] = _build_program(T, has_bias=has_bias)
    return _NC_CACHE[key]


def _gate_perm():
    # TF order [i, j, f, o] (256 each) -> [f, i, o, j]
    i = np.arange(0, 256)
    j = np.arange(256, 512)
    f = np.arange(512, 768)
    o = np.arange(768, 1024)
    return np.concatenate([f, i, o, j])


def _prep_lstm_w(W, b, cap_table, perm, layer0, has_bias):
    """Gate-permute, fold cap_table (layer 0) and forget bias, add bias row.

    When has_bias is False the +1.0 forget bias is applied on-device via the
    ScalarE activation bias, and layers 1/2 carry no bias row at all."""
    Wp = np.asarray(W, np.float32)[:, perm]
    bp = np.asarray(b, np.float32)[perm].copy()
    if has_bias:
        bp[0:256] += 1.0  # forget_bias folded into the sigmoid argument
    if layer0:
        w_emb = Wp[0:200]
        w_cap = np.asarray(cap_table, np.float32) @ Wp[200:203]  # [4, 1024]
        w_h = Wp[203:459]
        stacked = np.concatenate(
            [w_emb[0:128], w_emb[128:200], w_cap, bp[None, :], w_h], axis=0)
        assert stacked.shape[0] == 461
    elif has_bias:
        stacked = np.concatenate([Wp[0:256], bp[None, :], Wp[256:512]], axis=0)
        assert stacked.shape[0] == 513
    else:
        stacked = Wp
        assert stacked.shape[0] == 512
    return stacked


def _to_bf16(x):
    import ml_dtypes
    return np.ascontiguousarray(np.asarray(x)).astype(ml_dtypes.bfloat16)


def kernel(**inputs):
    import time as _time
    _tprep = _time.time()
    words = np.asarray(inputs["words"])
    capitals = np.asarray(inputs["capitals"])
    B, T = words.shape
    assert B == B_FULL

    perm = _gate_perm()
    cap_table = np.asarray(inputs["cap_table"], np.float32)
    # biases of layers 1/2 are usually all-zero; then the only bias is the
    # +1.0 forget bias, applied for free via the ScalarE activation bias,
    # and the per-step bias matmuls are dropped entirely
    hb = any(np.any(np.asarray(inputs[k], np.float32) != 0.0)
             for k in ("bf1", "bf2", "bb1", "bb2"))
    nc = _get_program(T, hb)

    w_by_dir = [
        [_prep_lstm_w(inputs["Wf0"], inputs["bf0"], cap_table, perm, True, hb),
         _prep_lstm_w(inputs["Wf1"], inputs["bf1"], cap_table, perm, False, hb),
         _prep_lstm_w(inputs["Wf2"], inputs["bf2"], cap_table, perm, False, hb)],
        [_prep_lstm_w(inputs["Wb0"], inputs["bb0"], cap_table, perm, True, hb),
         _prep_lstm_w(inputs["Wb1"], inputs["bb1"], cap_table, perm, False, hb),
         _prep_lstm_w(inputs["Wb2"], inputs["bb2"], cap_table, perm, False, hb)],
    ]
    w_bf = [[_to_bf16(w) for w in ws] for ws in w_by_dir]

    d1w_np = _to_bf16(inputs["d1_W"])
    d1b_np = _to_bf16(np.asarray(inputs["d1_b"])[None, :])
    d2w_np = np.ascontiguousarray(np.asarray(inputs["d2_W"], np.float32))
    d2b_np = np.ascontiguousarray(np.asarray(inputs["d2_b"], np.float32)[None, :])

    # ---- per-core index/cap prep; emb table uploaded bf16, 1/8 per core ----
    import ml_dtypes
    emb_bf = np.asarray(inputs["embed_words"]).astype(ml_dtypes.bfloat16)
    VSH = VOCAB // 8
    ones_row = np.ones((1, BQ * T), np.float32)

    in_maps = []
    for p in range(8):
        d, q = p // 4, p % 4
        wl = words[BQ * q:BQ * (q + 1)]
        cl = capitals[BQ * q:BQ * (q + 1)]
        if d == 1:
            wl = wl[:, ::-1]
            cl = cl[:, ::-1]
        # t-major token order r = t*BQ + b, fed as [128, NTILE], token = 128j+p
        wflat = np.ascontiguousarray(wl.T).reshape(-1)
        ntile = wflat.shape[0] // 128
        widx_np = np.ascontiguousarray(
            wflat.reshape(ntile, 128).T).astype(np.int32)
        cflat = np.ascontiguousarray(cl.T).reshape(-1)
        caph_np = np.concatenate(
            [(cflat[None, :] == np.arange(4)[:, None]).astype(np.float32),
             ones_row], axis=0).astype(ml_dtypes.bfloat16)
        in_maps.append({
            "embsh": emb_bf[VSH * p:VSH * (p + 1)],
            "widx": widx_np,
            "caph": caph_np,
            "w0": w_bf[d][0],
            "w1": w_bf[d][1],
            "w2": w_bf[d][2],
            "d1w": d1w_np,
            "d1b": d1b_np,
            "d2w": d2w_np,
            "d2b": d2b_np,
        })

    global LAST_RESULTS, LAST_RUN_WALL_S, LAST_PREP_S
    LAST_PREP_S = _time.time() - _tprep
    kwargs = {}
    if TRACE:
        kwargs = dict(trace=True, trace_cores=list(range(8)))
    _t0 = _time.time()
    try:
        res = run_bass_kernel_spmd(nc, in_maps, core_ids=list(range(8)), **kwargs)
    except Exception:
        if not kwargs:
            raise
        res = run_bass_kernel_spmd(nc, in_maps, core_ids=list(range(8)))
    LAST_RUN_WALL_S = _time.time() - _t0
    LAST_RESULTS = res
    return np.ascontiguousarray(res.results[0]["out"].T.astype(np.float32))


# revision 24
# speedup vs baseline: 3.9889x; 1.2443x over previous
"""BiLSTM Trainium2 kernel.

Sharding: 8 cores = 4 batch quarters x 2 directions.
  core p: direction d = p // 4 (0=fwd, 1=bwd), batch quarter q = p % 4
  (the backward direction is the forward LSTM run on a time-reversed
  sequence; the final reduction is a max over time, which is order-invariant,
  so all 8 cores run the identical program on different data.)

Per core: 3 stacked LSTM layers over T steps, batch 32, H=256, run as a
lag-1 wavefront (layer l processes step t = tick - l), fully SBUF-resident:
  - token embeddings gathered + feature-major transposed on the host (bf16)
    and DMA'd straight into the X^T SBUF buffer (uploading the gathered
    activations [205 x 16000]x2B per core beats shipping the 40MB embedding
    table to every core and gathering on-device)
  - the steady-state ticks run in a hardware For_i loop (unroll 6) instead
    of a fully unrolled instruction stream: the program drops from ~21k to
    <1k instructions, which cuts the per-call BIR serialization + walrus
    compile from ~1.4s to ~0.2s (the wall-clock metric includes them)
  - per tick: stage the tick's X^T column block to a static buffer (DVE copy
    with a register offset; the PE's LDWEIGHTS path cannot take register
    offsets) -> matmuls (weights streaming, batch-on-partition, fp32 PSUM
    accum) -> fused sigmoid/tanh on ScalarE across all three layers
    -> DVE cell-state update -> tanh(c) -> h -> PE transpose of h into
    feature-major h^T (the lhsT of the next tick's matmuls, ping-pong pair)
  - running max over t of layer-2 h^T
Final dense layers run on every core after an AllGather of the per-core maxes;
the host takes core 0's output.

Gate columns are permuted on host from TF order [i,j,f,o] to [f,i,o,j] so a
single ScalarE sigmoid covers all three sigmoid gates; when the layer-1/2
biases are all zero (the usual case) the +1.0 forget bias is applied for free
via the ScalarE activation-bias field and no per-step bias matmuls are
emitted; otherwise biases ride in an extra weight row against a ones-vector.
cap_table is folded into the layer-0 weights (one-hot @ (cap_table @ W_cap)).
"""

import sys

import numpy as np

sys.path.insert(0, "/opt/trn_rl_repo")

from contextlib import ExitStack

import concourse.bacc as bacc
import concourse.bass as bass
import concourse.mybir as mybir
import concourse.tile as tile
from concourse.bass import ds, IndirectOffsetOnAxis
from concourse.bass_utils import run_bass_kernel_spmd
from concourse.masks import make_identity

FP32 = mybir.dt.float32
BF16 = mybir.dt.bfloat16
INT32 = mybir.dt.int32

VOCAB, EMB, T_FULL, B_FULL, H, NC_OUT = 50000, 200, 500, 128, 256, 6
BQ = 32          # batch per core
G4 = 4 * H       # 1024 gate width
HALF = 512       # matmul N per PSUM bank
UNROLL = 6       # steady-state ticks per hardware-loop body

# gate slices after host permutation [f, i, o, j]
SL_F = slice(0, 256)
SL_I = slice(256, 512)
SL_O = slice(512, 768)
SL_J = slice(768, 1024)


def _build_program(T, with_tail=True, has_bias=True):
    """Build the single SPMD Bass program (same for every core)."""
    TOK = BQ * T                      # tokens per core
    assert (T - 2) % UNROLL == 0 and T % 2 == 0

    nc = bacc.Bacc(None, target_bir_lowering=False, debug=False)

    # ---- external inputs (per-core data) ----
    NTILE = TOK // 128                # 128-token gather tiles
    assert TOK % 128 == 0 and VOCAB % 8 == 0
    VSH = VOCAB // 8                  # vocab shard per core
    embsh = nc.dram_tensor("embsh", [VSH, EMB], BF16, kind="ExternalInput")
    widx = nc.dram_tensor("widx", [128, NTILE], INT32, kind="ExternalInput")
    caph = nc.dram_tensor("caph", [5, TOK], BF16, kind="ExternalInput")
    # LSTM weights ride in a direction-blob: rows [w0 | w1 | w2 | d1w],
    # zero-padded to a multiple of 4, and each core uploads 1/4 of its
    # direction's blob; an AllGather over replica groups [[0..3],[4..7]]
    # reassembles the right blob on every core with no partition-id logic.
    wrows = 513 if has_bias else 512
    WB_ROWS = 461 + 2 * wrows + 32          # + d1w as [32, 1024]
    WB_PAD = (-WB_ROWS) % 4
    WB = WB_ROWS + WB_PAD
    wsh = nc.dram_tensor("wsh", [WB // 4, G4], BF16, kind="ExternalInput")
    d1b = nc.dram_tensor("d1b", [1, 64], BF16, kind="ExternalInput")
    d2w = nc.dram_tensor("d2w", [64, NC_OUT], FP32, kind="ExternalInput")
    d2b = nc.dram_tensor("d2b", [1, NC_OUT], FP32, kind="ExternalInput")
    out = nc.dram_tensor("out", [NC_OUT, B_FULL], FP32, kind="ExternalOutput")

    with tile.TileContext(nc) as tc, ExitStack() as ctx:
        const = ctx.enter_context(tc.tile_pool(name="const", bufs=1))
        wpool = ctx.enter_context(tc.tile_pool(name="wpool", bufs=1))
        xtp = ctx.enter_context(tc.tile_pool(name="xtp", bufs=1))
        state = ctx.enter_context(tc.tile_pool(name="state", bufs=1))
        gpool = ctx.enter_context(tc.tile_pool(name="gpool", bufs=3))
        zg = ctx.enter_context(tc.tile_pool(name="zg", bufs=3))
        hpool = ctx.enter_context(tc.tile_pool(name="hpool", bufs=2))
        dram = ctx.enter_context(tc.tile_pool(name="dram", bufs=1, space="DRAM"))

        # ---- constants ----
        id_f32 = const.tile([128, 128], FP32)
        make_identity(nc, id_f32[:])
        id_bf = const.tile([128, 128], BF16)
        nc.vector.tensor_copy(id_bf[:], id_f32[:])
        ones_bf = const.tile([1, 128], BF16)
        nc.gpsimd.memset(ones_bf[:], 1.0)
        ones_f32 = const.tile([1, 128], FP32)
        nc.gpsimd.memset(ones_f32[:], 1.0)

        # ---- AllGather the direction weight blob; load into SBUF ----
        wsh_int = dram.tile([WB // 4, G4], BF16)
        nc.sync.dma_start(wsh_int[:], wsh[:, :])
        wblob = dram.tile([WB, G4], BF16)
        nc.gpsimd.collective_compute(
            "AllGather",
            mybir.AluOpType.bypass,
            replica_groups=[[0, 1, 2, 3], [4, 5, 6, 7]],
            ins=[wsh_int[:].opt()],
            outs=[wblob[:].opt()],
        )

        _wofs = [0]

        def load_w(nm, rows_chunks):
            tiles = []
            for i, rs in enumerate(rows_chunks):
                r0 = _wofs[0]
                t = wpool.tile([rs, G4], BF16, name=f"wt_{nm}_{i}")
                nc.sync.dma_start(t[:], wblob[r0:r0 + rs, :])
                tiles.append(t)
                _wofs[0] = r0 + rs
            return tiles

        w0a, w0b, w0c, w0d = load_w("w0", [128, 77, 128, 128])
        if has_bias:
            w1a, w1b, w1bias, w1c, w1d = load_w("w1", [128, 128, 1, 128, 128])
            w2a, w2b, w2bias, w2c, w2d = load_w("w2", [128, 128, 1, 128, 128])
        else:
            w1a, w1b, w1c, w1d = load_w("w1", [128, 128, 128, 128])
            w2a, w2b, w2c, w2d = load_w("w2", [128, 128, 128, 128])
            w1bias = w2bias = None

        # d1w [512, 64] rides in the blob as [32, 1024]: blob row 16i+j/64
        # holds d1w rows 16i..16i+15 -> chunk c is blob rows [d1w0+32c/16 ...]
        d1w0 = _wofs[0]
        d1w_sb = []
        for c in range(4):
            t = wpool.tile([128, 64], BF16, name=f"d1w_{c}")
            src = wblob[d1w0 + 8 * c:d1w0 + 8 * (c + 1), :].rearrange(
                "a (b c) -> (a b) c", c=64)
            nc.sync.dma_start(t[:], src)
            d1w_sb.append(t)
        d1b_sb = wpool.tile([1, 64], BF16)
        nc.sync.dma_start(d1b_sb[:], d1b[:, :])
        d2w_sb = wpool.tile([64, NC_OUT], FP32)
        nc.sync.dma_start(d2w_sb[:], d2w[:, :])
        d2b_sb = wpool.tile([1, NC_OUT], FP32)
        nc.sync.dma_start(d2b_sb[:], d2b[:, :])

        # ---- recurrent state ----
        c_all = state.tile([96, H], FP32)       # cell state, 3 layers x 32 batch
        nc.gpsimd.memset(c_all[:], 0.0)
        maxht = state.tile([128, 2, BQ], BF16)  # running max of layer-2 h^T
        nc.gpsimd.memset(maxht[:], -10.0)
        # h^T ping-pong: tick tau writes ht_buf[tau % 2], reads ht_buf[1 - tau % 2]
        ht_buf = [state.tile([128, 2, 96], BF16, name=f"htbuf{p}")
                  for p in range(2)]
        nc.gpsimd.memset(ht_buf[1][:], 0.0)     # initial state read by tick 0

        # X^T: xt_a rows = emb features 0:128
        #      xt_b rows = emb features 128:200 (72) | cap one-hot (4) | ones (1)
        # Each core uploads a 1/8 vocab shard (2.5MB instead of the 6.5MB
        # pre-gathered activations); an on-device AllGather reassembles the
        # full bf16 table in DRAM and a hardware-looped indirect-DMA gather
        # + PE transpose builds X^T on-core. Upload is the wall-clock
        # bottleneck, on-device gather is ~free.
        xt_a = xtp.tile([128, TOK], BF16)
        xt_b = xtp.tile([77, TOK], BF16)
        nc.sync.dma_start(xt_b[72:77, :], caph[:, :])

        widx_sb = const.tile([128, NTILE], INT32)
        nc.sync.dma_start(widx_sb[:], widx[:, :])

        embsh_int = dram.tile([VSH, EMB], BF16)
        nc.sync.dma_start(embsh_int[:], embsh[:, :])
        emb_full = dram.tile([VOCAB, EMB], BF16, addr_space="Shared")
        nc.gpsimd.collective_compute(
            "AllGather",
            mybir.AluOpType.bypass,
            replica_groups=[list(range(8))],
            ins=[embsh_int[:].opt()],
            outs=[emb_full[:].opt()],
        )

        GU = 5                         # gather-loop unroll
        assert NTILE % GU == 0
        with tc.tile_pool(name="pprep", bufs=2, space="PSUM") as pprep:
            widx_stg = [const.tile([128, 1], INT32, name=f"wix{p}")
                        for p in range(GU)]
            with tc.For_i(0, NTILE, GU) as j:
                for p in range(GU):
                    nc.vector.tensor_copy(widx_stg[p][:],
                                          widx_sb[:, ds(j + p, 1)])
                    g = gpool.tile([128, EMB], BF16, name="gemb", tag="gemb")
                    nc.gpsimd.indirect_dma_start(
                        out=g[:],
                        out_offset=None,
                        in_=emb_full[:],
                        in_offset=IndirectOffsetOnAxis(ap=widx_stg[p][:],
                                                       axis=0),
                    )
                    tp1 = pprep.tile([128, 128], BF16, name="tp1", tag="tp")
                    nc.tensor.transpose(tp1[:], g[:, 0:128], id_bf[:])
                    nc.vector.tensor_copy(
                        xt_a[:, ds(j * 128 + p * 128, 128)], tp1[:])
                    tp2 = pprep.tile([72, 128], BF16, name="tp2", tag="tp")
                    nc.tensor.transpose(tp2[:], g[:, 128:200], id_bf[:])
                    nc.vector.tensor_copy(
                        xt_b[0:72, ds(j * 128 + p * 128, 128)], tp2[:])

        # per-body-position staging for the current tick's X^T column block
        xstg = [(xtp.tile([128, BQ], BF16, name=f"xsa{p}"),
                 xtp.tile([77, BQ], BF16, name=f"xsb{p}"))
                for p in range(UNROLL)]

        with tc.tile_pool(name="pz", bufs=2, space="PSUM") as pz, \
             tc.tile_pool(name="pht", bufs=2, space="PSUM") as pht:

            def emit_tick(xa, xb, ht_prev, ht_cur, lo, hi, zero_tail,
                          maxupd):
                """One wavefront tick: layers lo..hi, layer l at step t=tau-l.

                xa/xb: lhsT for layer 0's x-part ([128,32] / [77,32] APs), or
                None when layer 0 is inactive.  ht_prev/ht_cur: h^T ping-pong
                buffers.  zero_tail: zero h rows above 32*(hi+1) (warmup).
                """
                lchunks = {}
                for l in range(lo, hi + 1):
                    if l == 0:
                        ch = [(xa, w0a), (xb, w0b),
                              (ht_prev[:, 0, 0:32], w0c),
                              (ht_prev[:, 1, 0:32], w0d)]
                    else:
                        wa, wb, wbias, wc, wd = (
                            (w1a, w1b, w1bias, w1c, w1d) if l == 1 else
                            (w2a, w2b, w2bias, w2c, w2d))
                        xs = slice(32 * (l - 1), 32 * l)
                        hs = slice(32 * l, 32 * (l + 1))
                        ch = [(ht_prev[:, 0, xs], wa),
                              (ht_prev[:, 1, xs], wb),
                              (ht_prev[:, 0, hs], wc),
                              (ht_prev[:, 1, hs], wd)]
                        if has_bias:
                            ch.insert(2, (ones_bf[0:1, 0:32], wbias))
                    lchunks[l] = ch

                z = pz.tile([96, G4], FP32, name="z", tag="z")
                maxk = max(len(v) for v in lchunks.values())
                for half in range(2):
                    ns = slice(HALF * half, HALF * (half + 1))
                    # interleave layers per chunk step: consecutive matmuls
                    # target different 32-col groups -> concurrent PE tiles
                    for k in range(maxk):
                        for l in range(lo, hi + 1):
                            chunks = lchunks[l]
                            if k >= len(chunks):
                                continue
                            lhsT, rhs = chunks[k]
                            zl = z[32 * l:32 * (l + 1), ns]
                            nc.tensor.matmul(
                                zl, lhsT, rhs[:, ns],
                                start=(k == 0), stop=(k == len(chunks) - 1),
                                skip_group_check=True)

                if lo == 0:
                    rlist = [slice(0, 32 * (hi + 1))]
                else:
                    rlist = [slice(32 * l, 32 * (l + 1))
                             for l in range(lo, hi + 1)]

                gates = zg.tile([96, G4], FP32, name="gates", tag="gates")
                t1 = zg.tile([96, H], FP32, name="t1", tag="t1")
                th = zg.tile([96, H], FP32, name="th", tag="th")
                h_all = hpool.tile([96, H], BF16, name="h_all", tag="h_all")
                for r in rlist:
                    if has_bias:
                        nc.scalar.activation(gates[r, 0:768], z[r, 0:768],
                                             mybir.ActivationFunctionType.Sigmoid)
                    else:
                        nc.scalar.activation(gates[r, SL_F], z[r, SL_F],
                                             mybir.ActivationFunctionType.Sigmoid,
                                             bias=1.0)
                        nc.scalar.activation(gates[r, 256:768], z[r, 256:768],
                                             mybir.ActivationFunctionType.Sigmoid)
                    nc.scalar.activation(gates[r, SL_J], z[r, SL_J],
                                         mybir.ActivationFunctionType.Tanh)
                    nc.vector.tensor_tensor(c_all[r], gates[r, SL_F], c_all[r],
                                            op=mybir.AluOpType.mult)
                    nc.vector.tensor_tensor(t1[r], gates[r, SL_I],
                                            gates[r, SL_J],
                                            op=mybir.AluOpType.mult)
                    nc.vector.tensor_tensor(c_all[r], c_all[r], t1[r],
                                            op=mybir.AluOpType.add)
                    nc.scalar.activation(th[r], c_all[r],
                                         mybir.ActivationFunctionType.Tanh)
                    nc.vector.tensor_tensor(h_all[r], gates[r, SL_O], th[r],
                                            op=mybir.AluOpType.mult)
                if zero_tail:
                    for rz in range(hi + 1, 3):
                        nc.vector.memset(h_all[32 * rz:32 * (rz + 1), :], 0.0)

                # transpose h into feature-major h^T (during warmup the zeroed
                # tail rows are transposed too so the buffer reads as the
                # correct zero initial state)
                if lo == 0 or zero_tail:
                    for c in range(2):
                        tp = pht.tile([128, 96], BF16, name="htpp", tag="htpp")
                        nc.tensor.transpose(tp[:],
                                            h_all[:, 128 * c:128 * (c + 1)],
                                            id_bf[0:96, 0:96])
                        nc.vector.tensor_copy(ht_cur[:, c, :], tp[:])
                else:
                    # epilogue: only layers lo..hi are live; 32-row groups
                    # (non-zero partition base must span <= 32)
                    for c in range(2):
                        for l in range(lo, hi + 1):
                            rs = slice(32 * l, 32 * (l + 1))
                            tp = pht.tile([128, 32], BF16, name="htpe",
                                          tag="htpp")
                            nc.tensor.transpose(tp[:],
                                                h_all[rs, 128 * c:128 * (c + 1)],
                                                id_bf[rs, rs])
                            nc.vector.tensor_copy(ht_cur[:, c, rs], tp[:])

                if maxupd:
                    nc.vector.tensor_tensor(maxht[:], maxht[:],
                                            ht_cur[:, :, 64:96],
                                            op=mybir.AluOpType.max)

            # ---- warmup ticks 0,1 (unrolled; static X^T slices) ----
            emit_tick(xt_a[:, 0:BQ], xt_b[:, 0:BQ],
                      ht_buf[1], ht_buf[0], 0, 0, True, False)
            emit_tick(xt_a[:, BQ:2 * BQ], xt_b[:, BQ:2 * BQ],
                      ht_buf[0], ht_buf[1], 0, 1, True, False)

            # ---- steady-state ticks 2..T-1 in a hardware loop ----
            # tau = 2 + i + p; i even so tau parity == p parity
            with tc.For_i(0, T - 2, UNROLL) as i:
                for p in range(UNROLL):
                    xa, xb = xstg[p]
                    nc.vector.tensor_copy(
                        xa[:], xt_a[:, ds(i * BQ + (2 + p) * BQ, BQ)])
                    nc.vector.tensor_copy(
                        xb[:], xt_b[:, ds(i * BQ + (2 + p) * BQ, BQ)])
                    emit_tick(xa[:], xb[:],
                              ht_buf[1 - p % 2], ht_buf[p % 2], 0, 2,
                              False, True)

            # ---- drain ticks T, T+1 (unrolled; layers 1..2 then 2) ----
            emit_tick(None, None, ht_buf[1], ht_buf[0], 1, 2, False, True)
            emit_tick(None, None, ht_buf[0], ht_buf[1], 2, 2, False, True)

        if not with_tail:
            # cost-model builds stop before the collective tail; keep maxht
            # live by dumping a slice to the output tensor
            nc.gpsimd.dma_start(out[0:6, 0:32], maxht[0:6, 0, :])
        else:
            # ---- AllGather of per-core maxes; dense head on every core ----
            tc.strict_bb_all_engine_barrier()
            mh_dram = dram.tile([128, 2 * BQ], BF16)
            nc.sync.dma_start(
                mh_dram[:].rearrange("p (c rr) -> p c rr", c=2), maxht[:, :, :])
            ag = dram.tile([8 * 128, 2 * BQ], BF16)
            nc.gpsimd.collective_compute(
                "AllGather",
                mybir.AluOpType.bypass,
                replica_groups=[list(range(8))],
                ins=[mh_dram[:].opt()],
                outs=[ag[:].opt()],
            )

            # rnn^T chunk (d2, c) [128, 128]: feature f = 256*d2 + 128*c + p,
            # batch b = 32*q + rr  ->  ag[(4*d2+q)*128 + p, c*32 + rr]
            tc.strict_bb_all_engine_barrier()
            agv = ag[:].rearrange("(g p) (c rr) -> g p c rr", p=128, c=2)
            rnn_chunks = []
            for d2 in range(2):
                for c in range(2):
                    rc = gpool.tile([128, 4, 32], BF16, name=f"rnn_{d2}_{c}",
                                    tag="rnn", bufs=4)
                    nc.sync.dma_start(
                        rc[:],
                        agv[4 * d2:4 * d2 + 4, :, c, :].rearrange("g p rr -> p g rr"))
                    rnn_chunks.append(rc)

            with tc.tile_pool(name="pdense", bufs=1, space="PSUM") as pdense:
                h1t = pdense.tile([64, B_FULL], FP32)
                for k in range(4):
                    nc.tensor.matmul(
                        h1t[:], d1w_sb[k][:],
                        rnn_chunks[k][:].rearrange("p g rr -> p (g rr)"),
                        start=(k == 0), stop=False, skip_group_check=True)
                nc.tensor.matmul(h1t[:], d1b_sb[:], ones_bf[:],
                                 start=False, stop=True, skip_group_check=True)

                # elu(x) = max(x,0) + exp(min(x,0)) - 1
                m = zg.tile([64, B_FULL], FP32, name="m", tag="m")
                nc.vector.tensor_scalar_min(m[:], h1t[:], 0.0)
                e = zg.tile([64, B_FULL], FP32, name="e", tag="m")
                nc.scalar.activation(e[:], m[:], mybir.ActivationFunctionType.Exp)
                h1f = zg.tile([64, B_FULL], FP32, name="h1f", tag="m")
                nc.vector.tensor_scalar_max(h1f[:], h1t[:], 0.0)
                nc.vector.tensor_tensor(h1f[:], h1f[:], e[:], op=mybir.AluOpType.add)
                nc.vector.tensor_scalar_add(h1f[:], h1f[:], -1.0)

                o_ps = pdense.tile([NC_OUT, B_FULL], FP32)
                nc.tensor.matmul(o_ps[:], d2w_sb[:], h1f[:], start=True, stop=False,
                                 skip_group_check=True)
                nc.tensor.matmul(o_ps[:], d2b_sb[:], ones_f32[:],
                                 start=False, stop=True, skip_group_check=True)
                o_sb = zg.tile([NC_OUT, B_FULL], FP32, name="o_sb", tag="m")
                nc.scalar.activation(o_sb[:], o_ps[:],
                                     mybir.ActivationFunctionType.Sigmoid)
                nc.sync.dma_start(out[:, :], o_sb[:])

    nc.finalize()
    return nc


_NC_CACHE = {}
TRACE = False
LAST_RESULTS = None
LAST_RUN_WALL_S = None
LAST_PREP_S = None


def _get_program(T, has_bias=True):
    key = (T, has_bias)
    if key not in _NC_CACHE:
        _NC_CACHE[key] = _build_program(T, has_bias=has_bias)
    return _NC_CACHE[key]


def _gate_perm():
    # TF order [i, j, f, o] (256 each) -> [f, i, o, j]
    i = np.arange(0, 256)
    j = np.arange(256, 512)
    f = np.arange(512, 768)
    o = np.arange(768, 1024)
    return np.concatenate([f, i, o, j])


def _prep_lstm_w(W, b, cap_table, perm, layer0, has_bias):
    """Gate-permute, fold cap_table (layer 0) and forget bias, add bias row.

    When has_bias is False the +1.0 forget bias is applied on-device via the
    ScalarE activation bias, and layers 1/2 carry no bias row at all."""
    Wp = np.asarray(W, np.float32)[:, perm]
    bp = np.asarray(b, np.float32)[perm].copy()
    if has_bias:
        bp[0:256] += 1.0  # forget_bias folded into the sigmoid argument
    if layer0:
        w_emb = Wp[0:200]
        w_cap = np.asarray(cap_table, np.float32) @ Wp[200:203]  # [4, 1024]
        w_h = Wp[203:459]
        stacked = np.concatenate(
            [w_emb[0:128], w_emb[128:200], w_cap, bp[None, :], w_h], axis=0)
        assert stacked.shape[0] == 461
    elif has_bias:
        stacked = np.concatenate([Wp[0:256], bp[None, :], Wp[256:512]], axis=0)
        assert stacked.shape[0] == 513
    else:
        stacked = Wp
        assert stacked.shape[0] == 512
    return stacked


def _to_bf16(x):
    import ml_dtypes
    return np.ascontiguousarray(np.asarray(x)).astype(ml_dtypes.bfloat16)


def kernel(**inputs):
    import time as _time
    _tprep = _time.time()
    words = np.asarray(inputs["words"])
    capitals = np.asarray(inputs["capitals"])
    B, T = words.shape
    assert B == B_FULL

    perm = _gate_perm()
    cap_table = np.asarray(inputs["cap_table"], np.float32)
    # biases of layers 1/2 are usually all-zero; then the only bias is the
    # +1.0 forget bias, applied for free via the ScalarE activation bias,
    # and the per-step bias matmuls are dropped entirely
    hb = any(np.any(np.asarray(inputs[k], np.float32) != 0.0)
             for k in ("bf1", "bf2", "bb1", "bb2"))
    nc = _get_program(T, hb)

    w_by_dir = [
        [_prep_lstm_w(inputs["Wf0"], inputs["bf0"], cap_table, perm, True, hb),
         _prep_lstm_w(inputs["Wf1"], inputs["bf1"], cap_table, perm, False, hb),
         _prep_lstm_w(inputs["Wf2"], inputs["bf2"], cap_table, perm, False, hb)],
        [_prep_lstm_w(inputs["Wb0"], inputs["bb0"], cap_table, perm, True, hb),
         _prep_lstm_w(inputs["Wb1"], inputs["bb1"], cap_table, perm, False, hb),
         _prep_lstm_w(inputs["Wb2"], inputs["bb2"], cap_table, perm, False, hb)],
    ]
    # direction weight blob: [w0 | w1 | w2 | d1w as [32,1024]], padded to %4
    d1w_flat = np.asarray(inputs["d1_W"], np.float32).reshape(32, 1024)
    wb_rows = 461 + 2 * (513 if hb else 512) + 32
    wb = wb_rows + (-wb_rows) % 4
    blob_by_dir = []
    for ws in w_by_dir:
        blob = np.concatenate(
            ws + [d1w_flat,
                  np.zeros((wb - wb_rows, 1024), np.float32)], axis=0)
        blob_by_dir.append(_to_bf16(blob))

    d1b_np = _to_bf16(np.asarray(inputs["d1_b"])[None, :])
    d2w_np = np.ascontiguousarray(np.asarray(inputs["d2_W"], np.float32))
    d2b_np = np.ascontiguousarray(np.asarray(inputs["d2_b"], np.float32)[None, :])

    # ---- per-core index/cap prep; emb table uploaded bf16, 1/8 per core ----
    import ml_dtypes
    emb_bf = np.asarray(inputs["embed_words"]).astype(ml_dtypes.bfloat16)
    VSH = VOCAB // 8
    ones_row = np.ones((1, BQ * T), np.float32)

    in_maps = []
    for p in range(8):
        d, q = p // 4, p % 4
        wl = words[BQ * q:BQ * (q + 1)]
        cl = capitals[BQ * q:BQ * (q + 1)]
        if d == 1:
            wl = wl[:, ::-1]
            cl = cl[:, ::-1]
        # t-major token order r = t*BQ + b, fed as [128, NTILE], token = 128j+p
        wflat = np.ascontiguousarray(wl.T).reshape(-1)
        ntile = wflat.shape[0] // 128
        widx_np = np.ascontiguousarray(
            wflat.reshape(ntile, 128).T).astype(np.int32)
        cflat = np.ascontiguousarray(cl.T).reshape(-1)
        caph_np = np.concatenate(
            [(cflat[None, :] == np.arange(4)[:, None]).astype(np.float32),
             ones_row], axis=0).astype(ml_dtypes.bfloat16)
        wq = blob_by_dir[d].shape[0] // 4
        in_maps.append({
            "embsh": emb_bf[VSH * p:VSH * (p + 1)],
            "widx": widx_np,
            "caph": caph_np,
            "wsh": blob_by_dir[d][wq * q:wq * (q + 1)],
            "d1b": d1b_np,
            "d2w": d2w_np,
            "d2b": d2b_np,
        })

    global LAST_RESULTS, LAST_RUN_WALL_S, LAST_PREP_S
    LAST_PREP_S = _time.time() - _tprep
    kwargs = {}
    if TRACE:
        kwargs = dict(trace=True, trace_cores=list(range(8)))
    _t0 = _time.time()
    try:
        res = run_bass_kernel_spmd(nc, in_maps, core_ids=list(range(8)), **kwargs)
    except Exception:
        if not kwargs:
            raise
        res = run_bass_kernel_spmd(nc, in_maps, core_ids=list(range(8)))
    LAST_RUN_WALL_S = _time.time() - _t0
    LAST_RESULTS = res
    return np.ascontiguousarray(res.results[0]["out"].T.astype(np.float32))
